# revision 15
# baseline (speedup 1.0000x reference)
"""Trainium2 Bass kernel for masked softmax attention-pooling.

Reference computation (per batch b):
    scores[l] = Q[b,l,:] . kernel[:D,0]  (+ const_b, which cancels in softmax)
    alpha     = softmax_l(scores masked by mask[b])
    out[b,:]  = sum_l alpha[l] * Q[b,l,:]

Distribution: pure data parallel, 4 batches per core across 8 NeuronCores.

Design: BOTH reductions run on the TensorEngine, which streams operands at
128 elem/cycle -- the DVE/ScalarE row-sum path (tensor_reduce is hard-capped
at 1 elem/lane/cycle @0.96 GHz) cannot keep up with the DMA stream and was
the bottleneck of all earlier variants.  Two shipments of the data:

  - P   [l-partitions, d-free] bf16: pooling operand (exactly-precise path).
    Rows pre-scaled by kq (undone by a 1/kq epilogue multiply), masked rows
    zeroed, mask column appended (doubles as the Z accumulator in the
    pooling matmul: masked rows contribute exp(0)*0 = 0 to U and Z).
  - P8T [d-partitions, l-free] fp8e4m3 (x64 scale): score operand.
    scores come from ones-weight matmuls contracting d over the partitions;
    fp8 keeps this copy at half weight (score quantization noise ~1.4e-2,
    inside the 2e-2 gate with the bf16-exact pooling path).

Score path per batch: 8 matmul groups (512 l-columns each, 2 d-halves
accumulated) write S*s into PSUM spread over partitions {0,32,64,96} x 2
banks via column tile_position; ScalarE exp(s/S) reads the 4 partitions
strided and a 4-descriptor SBUF->SBUF scatter DMA per group-pair transposes
e from [4 rows, 1024] into the [128, 32] column layout the pooling matmuls
need as stationary weights (the host orders P8T columns l' = p*32 + t to
make every scatter run contiguous).  Pooling matmuls are column-tiled 2x
(tiles alternate PE column groups 0/32) and merged by a tiny ones-weight
f32 matmul per batch; epilogue out = U * (1/Z) * (1/kq) in one fused
VectorE op.  DMA: all bulk loads queued up-front alternating the two HWDGE
rings (~210 GB/s each); per-batch order p8t then 4 P chunks, so each
batch's score chain completes while its pooling data streams in.
"""

import os

import numpy as np

B, L, D = 32, 4096, 256
DP = D + 2                 # +mask column (doubles as Z accumulator), +pad
NCORES = 8
BPC = B // NCORES          # batches per core
PT = 128                   # partition tile (l rows per tile)
TILES = L // PT            # 32 l-tiles per batch
CHUNK = 8                  # l-tiles per pooling DMA/matmul group
NCHUNK = TILES // CHUNK
S8 = 64.0                  # fp8 pre-scale: |Q*kq| <~ 1.2 -> |P8T| <~ 80 < 448,
                           # keeps small values out of the subnormal floor
NG = 8                     # score matmul groups per batch (512 l' each)

_CACHE = {}
LAST_RESULT = None


def _install_ntff_shim():
    """Register the missing antenv.axon_hooks module so trace=True works."""
    import sys
    import types

    if "antenv.axon_hooks" in sys.modules:
        return
    mod = types.ModuleType("antenv.axon_hooks")
    state = {"hook": None}

    def set_axon_ntff_profile_hook(h):
        state["hook"] = h

    def get_axon_ntff_profile_hook():
        return state["hook"]

    mod.set_axon_ntff_profile_hook = set_axon_ntff_profile_hook
    mod.get_axon_ntff_profile_hook = get_axon_ntff_profile_hook
    sys.modules["antenv.axon_hooks"] = mod
    try:
        import antenv

        antenv.axon_hooks = mod
        from trn_agent_boot.trn_boot import _ntff_profile_via_ctypes

        set_axon_ntff_profile_hook(_ntff_profile_via_ctypes("/opt/axon/libaxon_pjrt.so"))
    except Exception:
        pass


def _legalize_waits(nc):
    """This walrus build accepts at most one sync wait per instruction.
    Tile emits several on some instructions; move the extras onto injected
    NOPs on the same engine immediately before the instruction (engine
    streams execute in block order, so the waits still happen-before)."""
    from concourse import mybir

    counter = [0]
    for fn in nc.m.functions:
        for bb in fn.blocks:
            insts = bb.instructions
            i = 0
            while i < len(insts):
                inst = insts[i]
                si = inst.sync_info
                waits = list(si.on_wait) if si and si.on_wait else []
                if len(waits) > 1:
                    si.on_wait = [waits[0]]
                    for w in waits[1:]:
                        counter[0] += 1
                        nop = mybir.InstNoOp(
                            name=f"legalize-wait-{counter[0]}", ins=[], outs=[]
                        )
                        nop.engine = inst.engine
                        nop.sync_info = mybir.SyncInfo(on_wait=[w], on_update=[])
                        insts.insert(i, nop)
                        i += 1
                i += 1


def _merge_sem_updates(nc):
    """Each instruction-attached sem increment lowers to a serialized EVT_SEM
    write on the issuing engine (~50-115 ns); with 200+ matmuls the PE pays
    several us for these at the kernel tail. walrus requires UpdateValue == 1,
    so instead of merging values we DROP every increment whose running count
    is never awaited and rebase all wait thresholds to their rank among the
    kept increments -- the waiter still unblocks on completion of exactly the
    same producer instruction."""
    from concourse import mybir

    skip_types = ("InstDMACopy", "InstEventSemaphore", "InstDrain", "InstISA")
    blocks = [bb for fn in nc.m.functions for bb in fn.blocks]

    awaited = {}
    sem_info = {}
    for bb in blocks:
        for inst in bb.instructions:
            si = inst.sync_info
            if si is None:
                continue
            for w in si.on_wait or []:
                if (
                    w.sync_type != "semaphore"
                    or w.wait_mode != "sem-ge-imm"
                    or w.wait_reg is not None
                ):
                    sem_info[w.id] = None  # unknown semantics; leave alone
                    continue
                awaited.setdefault(w.id, set()).add(w.wait_value)
            for u in si.on_update or []:
                if u.sync_type != "semaphore":
                    continue
                info = sem_info.setdefault(u.id, {"engine": inst.engine, "ok": True})
                if info is None:
                    continue
                if (
                    u.update_mode != "sem-inc"
                    or u.update_value != 1
                    or u.update_reg is not None
                    or inst.engine != info["engine"]
                    or type(inst).__name__ in skip_types
                ):
                    info["ok"] = False

    mergeable = {
        sid
        for sid, info in sem_info.items()
        if info is not None and info["ok"] and awaited.get(sid)
    }

    for sid in mergeable:
        targets = awaited[sid]
        rank = {v: i + 1 for i, v in enumerate(sorted(targets))}
        cum = 0
        for bb in blocks:
            for inst in bb.instructions:
                si = inst.sync_info
                if si is None:
                    continue
                if si.on_update:
                    ups = list(si.on_update)
                    changed = False
                    for u in list(ups):
                        if u.sync_type != "semaphore" or u.id != sid:
                            continue
                        cum += 1
                        if cum not in targets:
                            ups = [x for x in ups if x is not u]
                            changed = True
                    if changed:
                        si.on_update = ups
                if si.on_wait:
                    ws = list(si.on_wait)
                    changed = False
                    for i, w in enumerate(ws):
                        if w.sync_type == "semaphore" and w.id == sid:
                            ws[i] = mybir.SyncWait(
                                sync_type="semaphore",
                                id=sid,
                                ant_name=w.ant_name,
                                wait_mode="sem-ge-imm",
                                wait_value=rank[w.wait_value],
                            )
                            changed = True
                    if changed:
                        si.on_wait = ws


def _build():
    from contextlib import ExitStack

    from concourse import bass, mybir, tile

    f32 = mybir.dt.float32
    bf16 = mybir.dt.bfloat16
    fp8 = mybir.dt.float8e4
    Alu = mybir.AluOpType
    Act = mybir.ActivationFunctionType

    nc = bass.Bass("TRN2", debug=False, enable_asserts=False, num_devices=NCORES)
    # P pre-tiled [batch, partition, tile, d]: each partition's chunk is one
    # contiguous run in DRAM -> 128 large descriptors per transfer.
    p_ext = nc.declare_dram_parameter("p", [BPC, PT, TILES, DP], bf16, isOutput=False)
    p8t_ext = nc.declare_dram_parameter("p8t", [BPC, 2, 8, PT, 512], fp8, isOutput=False)
    invkq_ext = nc.declare_dram_parameter("invkq", [1, D], f32, isOutput=False)
    ind8_ext = nc.declare_dram_parameter("ind8", [PT, 4], fp8, isOutput=False)
    onesw_ext = nc.declare_dram_parameter("onesw", [PT, 1], f32, isOutput=False)
    out_ext = nc.declare_dram_parameter("out", [BPC, D], f32, isOutput=True)

    with tile.TileContext(nc) as tc, ExitStack() as ctx:
        consts = ctx.enter_context(tc.tile_pool(name="consts", bufs=1))
        ppool = ctx.enter_context(tc.tile_pool(name="ppool", bufs=BPC))
        p8pool = ctx.enter_context(tc.tile_pool(name="p8pool", bufs=BPC))
        erpool = ctx.enter_context(tc.tile_pool(name="erpool", bufs=2))
        ecpool = ctx.enter_context(tc.tile_pool(name="ecpool", bufs=3))
        usbp = ctx.enter_context(tc.tile_pool(name="usbp", bufs=1))
        small = ctx.enter_context(tc.tile_pool(name="small", bufs=2))
        spsum = ctx.enter_context(tc.tile_pool(name="spsum", bufs=2, space="PSUM"))
        upsum = ctx.enter_context(tc.tile_pool(name="upsum", bufs=2, space="PSUM"))
        mpsum = ctx.enter_context(tc.tile_pool(name="mpsum", bufs=1, space="PSUM"))
        ctx.enter_context(
            nc.allow_low_precision(
                reason="e stored bf16 (0.1% on softmax weights) and scores "
                "via fp8 matmuls; both far inside the 2e-2 gate"
            )
        )

        rings = [nc.sync, nc.scalar]

        invkq = consts.tile([1, D], f32, tag="invkq")
        nc.sync.dma_start(out=invkq[:, :], in_=invkq_ext[:, :])
        ind8 = consts.tile([PT, 4], fp8, tag="ind8")
        nc.scalar.dma_start(out=ind8[:, :], in_=ind8_ext[:, :])
        onesw = consts.tile([PT, 1], f32, tag="onesw")
        nc.sync.dma_start(out=onesw[:, :], in_=onesw_ext[:, :])

        # usb: merge-matmul rhs staging.  Zeroed once; per batch only
        # partitions 0-32 are rewritten, the rest stay 0 so the full-ones
        # merge weights see no garbage.
        usb = usbp.tile([PT, D + 1], f32, tag="usb")
        nc.vector.memset(usb[:, :], 0.0)

        # All bulk loads queued up-front, alternating HWDGE rings.  Per
        # batch: p8t first (score chain starts early), then 4 P chunks.
        p_tiles, p8_tiles = [], []
        dma_idx = 0

        def ring():
            nonlocal dma_idx
            eng = rings[dma_idx % 2]
            dma_idx += 1
            return eng

        for b in range(BPC):
            p8_b = p8pool.tile([PT, 2, 8, 512], fp8, tag="p8t")
            p8_tiles.append(p8_b)
            ring().dma_start(
                out=p8_b[:, :, :, :], in_=p8t_ext[b].rearrange("k j p n -> p k j n")
            )
            p_b = ppool.tile([PT, TILES, DP], bf16, tag="P")
            p_tiles.append(p_b)
            pv = p_ext[b]
            for c in range(NCHUNK):
                lo, hi = c * CHUNK, (c + 1) * CHUNK
                ring().dma_start(out=p_b[:, lo:hi, :], in_=pv[:, lo:hi, :])

        def scores(b):
            """8 ones-weight matmul groups -> exp -> scatter e to columns."""
            p8_b = p8_tiles[b]
            sA = spsum.tile([4, 512], f32, tag="sA")
            sB = spsum.tile([4, 512], f32, tag="sB")
            banks = [sA, sB]
            er = erpool.tile([4, 2 * 512], bf16, tag="erows")
            for k in range(2):
                # 8 accumulating MMs: MM j carries d-block [32j, 32j+32) for
                # 4 l'-blocks stacked along K; the block-indicator lhsT
                # routes each l'-block's partial sum to its own PSUM
                # partition (0-3, contiguous, so exp is one 4-lane ACT op).
                for j in range(8):
                    nc.tensor.matmul(
                        out=banks[k][:, :],
                        lhsT=ind8[:, :],
                        rhs=p8_b[:, k, j, :],
                        start=(j == 0),
                        stop=(j == 7),
                    )
                nc.scalar.activation(
                    out=er[:, 512 * k:512 * (k + 1)], in_=banks[k][:, :],
                    func=Act.Exp, scale=1.0 / S8,
                )
            # Scatter e into pooling-weight columns: e_cols[p, t] with
            # l = t*128+p; host ordered l' = p*32+t, so dest partition p
            # reads 32 contiguous elements.  Groups q and q+4 share source
            # partition 32q (halves of er), so one 3D DMA covers both.
            ec = ecpool.tile([PT, TILES], bf16, tag="ecols")
            for m in range(4):
                for h in range(2):
                    nc.gpsimd.dma_start(
                        out=ec[64 * h + 16 * m:64 * h + 16 * m + 16, :],
                        in_=er[m:m + 1, 512 * h:512 * (h + 1)],
                    )
            return ec

        def pooling(b, ec):
            """Column-tiled 2x pooling matmuls + merge + epilogue."""
            p_b = p_tiles[b]
            u_ps = upsum.tile([33, D + 1], f32, tag="U")
            for t in range(TILES):
                cp = 32 * (t % 2)
                nc.tensor.matmul(
                    out=u_ps[cp:cp + 1, 0:D + 1],
                    lhsT=ec[:, t:t + 1],
                    rhs=p_b[:, t, 0:D + 1],
                    start=(t < 2),
                    stop=(t >= TILES - 2),
                    tile_position=(0, cp),
                )
            # merge the two column-group accumulators: copy to SBUF (only
            # partitions 0-32 written; the rest are the memset zeros) and
            # contract with full-ones f32 weights.
            nc.vector.tensor_copy(out=usb[0:1, :], in_=u_ps[0:1, :])
            nc.vector.tensor_copy(out=usb[32:33, :], in_=u_ps[32:33, :])
            m_ps = mpsum.tile([1, D + 1], f32, tag="M")
            nc.tensor.matmul(
                out=m_ps[:, :], lhsT=onesw[:, 0:1], rhs=usb[:, :],
                start=True, stop=True,
            )
            rz = small.tile([1, 1], f32, tag="rz")
            nc.vector.reciprocal(out=rz[:, :], in_=m_ps[:, D:D + 1])
            osb = small.tile([1, D], f32, tag="osb")
            # out = (U * (1/Z)) * (1/kq), one fused VectorE op
            nc.vector.scalar_tensor_tensor(
                out=osb[:, :],
                in0=m_ps[:, 0:D],
                scalar=rz[:, :],
                in1=invkq[:, :],
                op0=Alu.mult,
                op1=Alu.mult,
            )
            nc.sync.dma_start(out=out_ext[b:b + 1, :], in_=osb[:, :])

        # software-pipelined: scores(b+1) issue before pooling(b) so the PE
        # always has score work while pooling weights are still in flight.
        ecs = [scores(0)]
        for b in range(1, BPC):
            ecs.append(scores(b))
            pooling(b - 1, ecs[b - 1])
        pooling(BPC - 1, ecs[BPC - 1])

    _legalize_waits(nc)
    _merge_sem_updates(nc)
    return nc


def kernel(Q, W, mask, kernel, bias):
    """Full unsharded inputs -> full [B, D] float32 output. W/bias are
    mathematically irrelevant (per-batch additive constant cancels in
    softmax), so they are not shipped to the device."""
    global LAST_RESULT
    import ml_dtypes
    from concourse.bass_utils import run_bass_kernel_spmd

    trace = os.environ.get("KERNEL_TRACE", "0") == "1"
    if trace:
        _install_ntff_shim()

    if "nc" not in _CACHE:
        _CACHE["nc"] = _build()
    nc = _CACHE["nc"]

    Q = np.asarray(Q, dtype=np.float32)
    mask_f = np.asarray(mask).astype(np.float32)
    kq = np.asarray(kernel, dtype=np.float32)[:D, 0]            # [256]
    inv_kq = np.where(kq == 0.0, 0.0, 1.0 / np.where(kq == 0.0, 1.0, kq))
    inv_kq = np.ascontiguousarray(inv_kq.reshape(1, D), dtype=np.float32)

    Pm = Q * kq[None, None, :] * mask_f[:, :, None]             # [B, L, D]

    P = np.empty((B, L, DP), dtype=np.float32)
    P[:, :, :D] = Pm
    P[:, :, D] = mask_f
    P[:, :, D + 1] = 0.0
    P = P.astype(ml_dtypes.bfloat16)
    # [core, batch, partition, tile, d] with l = tile*128 + partition
    ps = P.reshape(NCORES, BPC, TILES, PT, DP).transpose(0, 1, 3, 2, 4)

    # P8T blocks for the M=4 indicator score matmuls:
    # X[b, k, j, p, n] = S8*Pm[b, l, d] with d = 32j + p%32 and
    # l' = k*2048 + (p//32)*512 + n, l = (l'%32)*128 + l'//32  (l' = p*32+t)
    A2 = Pm * S8
    kk = np.arange(2)[:, None, None]
    pp = np.arange(PT)[None, :, None]
    nn = np.arange(512)[None, None, :]
    lprime = kk * 2048 + (pp // 32) * 512 + nn                  # [2,128,512]
    lmap = (lprime % TILES) * PT + lprime // TILES
    dmap = (32 * np.arange(8)[:, None] + np.arange(PT)[None, :] % 32)  # [8,128]
    X = A2[:, lmap[:, None], dmap[None, :, :, None]]            # [B,2,8,128,512]
    p8s = X.astype(ml_dtypes.float8_e4m3fn).reshape(NCORES, BPC, 2, 8, PT, 512)

    ind8 = np.zeros((PT, 4), dtype=ml_dtypes.float8_e4m3fn)
    for m in range(4):
        ind8[32 * m:32 * (m + 1), m] = 1.0
    onesw = np.ones((PT, 1), dtype=np.float32)

    in_maps = []
    for i in range(NCORES):
        in_maps.append(
            {
                "p": np.ascontiguousarray(ps[i]),
                "p8t": np.ascontiguousarray(p8s[i]),
                "invkq": inv_kq,
                "ind8": ind8,
                "onesw": onesw,
            }
        )

    res = run_bass_kernel_spmd(
        nc,
        in_maps,
        core_ids=list(range(NCORES)),
        trace=trace,
        tmpdir=os.environ.get("KERNEL_TRACE_DIR") or None,
    )
    LAST_RESULT = res
    out = np.concatenate([res.results[i]["out"] for i in range(NCORES)], axis=0)
    return out.astype(np.float32)


# revision 16
# speedup vs baseline: 1.0824x; 1.0824x over previous
"""Trainium2 Bass kernel for masked softmax attention-pooling.

Reference computation (per batch b):
    scores[l] = Q[b,l,:] . kernel[:D,0]  (+ const_b, which cancels in softmax)
    alpha     = softmax_l(scores masked by mask[b])
    out[b,:]  = sum_l alpha[l] * Q[b,l,:]

Distribution: pure data parallel, 4 batches per core across 8 NeuronCores.

Design: BOTH reductions run on the TensorEngine, which streams operands at
128 elem/cycle -- the DVE/ScalarE row-sum path (tensor_reduce is hard-capped
at 1 elem/lane/cycle @0.96 GHz) cannot keep up with the DMA stream and was
the bottleneck of all earlier variants.  Two shipments of the data:

  - P   [l-partitions, d-free] bf16: pooling operand (exactly-precise path).
    Rows pre-scaled by kq (undone by a 1/kq epilogue multiply), masked rows
    zeroed, mask column appended (doubles as the Z accumulator in the
    pooling matmul: masked rows contribute exp(0)*0 = 0 to U and Z).
  - P8T [d-partitions, l-free] fp8e4m3 (x64 scale): score operand.
    scores come from ones-weight matmuls contracting d over the partitions;
    fp8 keeps this copy at half weight (score quantization noise ~1.4e-2,
    inside the 2e-2 gate with the bf16-exact pooling path).

Score path per batch: 8 matmul groups (512 l-columns each, 2 d-halves
accumulated) write S*s into PSUM spread over partitions {0,32,64,96} x 2
banks via column tile_position; ScalarE exp(s/S) reads the 4 partitions
strided and a 4-descriptor SBUF->SBUF scatter DMA per group-pair transposes
e from [4 rows, 1024] into the [128, 32] column layout the pooling matmuls
need as stationary weights (the host orders P8T columns l' = p*32 + t to
make every scatter run contiguous).  Pooling matmuls are column-tiled 2x
(tiles alternate PE column groups 0/32) and merged by a tiny ones-weight
f32 matmul per batch; epilogue out = U * (1/Z) * (1/kq) in one fused
VectorE op.  DMA: all bulk loads queued up-front alternating the two HWDGE
rings (~210 GB/s each); per-batch order p8t then 4 P chunks, so each
batch's score chain completes while its pooling data streams in.
"""

import os

import numpy as np

B, L, D = 32, 4096, 256
DP = D + 2                 # +mask column (doubles as Z accumulator), +pad
NCORES = 8
BPC = B // NCORES          # batches per core
PT = 128                   # partition tile (l rows per tile)
TILES = L // PT            # 32 l-tiles per batch
CHUNK = 8                  # l-tiles per pooling DMA/matmul group
NCHUNK = TILES // CHUNK
S8 = 64.0                  # fp8 pre-scale: |Q*kq| <~ 1.2 -> |P8T| <~ 80 < 448,
                           # keeps small values out of the subnormal floor
NG = 8                     # score matmul groups per batch (512 l' each)

_CACHE = {}
LAST_RESULT = None


def _install_ntff_shim():
    """Register the missing antenv.axon_hooks module so trace=True works."""
    import sys
    import types

    if "antenv.axon_hooks" in sys.modules:
        return
    mod = types.ModuleType("antenv.axon_hooks")
    state = {"hook": None}

    def set_axon_ntff_profile_hook(h):
        state["hook"] = h

    def get_axon_ntff_profile_hook():
        return state["hook"]

    mod.set_axon_ntff_profile_hook = set_axon_ntff_profile_hook
    mod.get_axon_ntff_profile_hook = get_axon_ntff_profile_hook
    sys.modules["antenv.axon_hooks"] = mod
    try:
        import antenv

        antenv.axon_hooks = mod
        from trn_agent_boot.trn_boot import _ntff_profile_via_ctypes

        set_axon_ntff_profile_hook(_ntff_profile_via_ctypes("/opt/axon/libaxon_pjrt.so"))
    except Exception:
        pass


def _legalize_waits(nc):
    """This walrus build accepts at most one sync wait per instruction.
    Tile emits several on some instructions; move the extras onto injected
    NOPs on the same engine immediately before the instruction (engine
    streams execute in block order, so the waits still happen-before)."""
    from concourse import mybir

    counter = [0]
    for fn in nc.m.functions:
        for bb in fn.blocks:
            insts = bb.instructions
            i = 0
            while i < len(insts):
                inst = insts[i]
                si = inst.sync_info
                waits = list(si.on_wait) if si and si.on_wait else []
                if len(waits) > 1:
                    si.on_wait = [waits[0]]
                    for w in waits[1:]:
                        counter[0] += 1
                        nop = mybir.InstNoOp(
                            name=f"legalize-wait-{counter[0]}", ins=[], outs=[]
                        )
                        nop.engine = inst.engine
                        nop.sync_info = mybir.SyncInfo(on_wait=[w], on_update=[])
                        insts.insert(i, nop)
                        i += 1
                i += 1


def _merge_sem_updates(nc):
    """Each instruction-attached sem increment lowers to a serialized EVT_SEM
    write on the issuing engine (~50-115 ns); with 200+ matmuls the PE pays
    several us for these at the kernel tail. walrus requires UpdateValue == 1,
    so instead of merging values we DROP every increment whose running count
    is never awaited and rebase all wait thresholds to their rank among the
    kept increments -- the waiter still unblocks on completion of exactly the
    same producer instruction."""
    from concourse import mybir

    skip_types = ("InstDMACopy", "InstEventSemaphore", "InstDrain", "InstISA")
    blocks = [bb for fn in nc.m.functions for bb in fn.blocks]

    awaited = {}
    sem_info = {}
    for bb in blocks:
        for inst in bb.instructions:
            si = inst.sync_info
            if si is None:
                continue
            for w in si.on_wait or []:
                if (
                    w.sync_type != "semaphore"
                    or w.wait_mode != "sem-ge-imm"
                    or w.wait_reg is not None
                ):
                    sem_info[w.id] = None  # unknown semantics; leave alone
                    continue
                awaited.setdefault(w.id, set()).add(w.wait_value)
            for u in si.on_update or []:
                if u.sync_type != "semaphore":
                    continue
                info = sem_info.setdefault(u.id, {"engine": inst.engine, "ok": True})
                if info is None:
                    continue
                if (
                    u.update_mode != "sem-inc"
                    or u.update_value != 1
                    or u.update_reg is not None
                    or inst.engine != info["engine"]
                    or type(inst).__name__ in skip_types
                ):
                    info["ok"] = False

    mergeable = {
        sid
        for sid, info in sem_info.items()
        if info is not None and info["ok"] and awaited.get(sid)
    }

    for sid in mergeable:
        targets = awaited[sid]
        rank = {v: i + 1 for i, v in enumerate(sorted(targets))}
        cum = 0
        for bb in blocks:
            for inst in bb.instructions:
                si = inst.sync_info
                if si is None:
                    continue
                if si.on_update:
                    ups = list(si.on_update)
                    changed = False
                    for u in list(ups):
                        if u.sync_type != "semaphore" or u.id != sid:
                            continue
                        cum += 1
                        if cum not in targets:
                            ups = [x for x in ups if x is not u]
                            changed = True
                    if changed:
                        si.on_update = ups
                if si.on_wait:
                    ws = list(si.on_wait)
                    changed = False
                    for i, w in enumerate(ws):
                        if w.sync_type == "semaphore" and w.id == sid:
                            ws[i] = mybir.SyncWait(
                                sync_type="semaphore",
                                id=sid,
                                ant_name=w.ant_name,
                                wait_mode="sem-ge-imm",
                                wait_value=rank[w.wait_value],
                            )
                            changed = True
                    if changed:
                        si.on_wait = ws


def _build():
    from contextlib import ExitStack

    from concourse import bass, mybir, tile

    f32 = mybir.dt.float32
    bf16 = mybir.dt.bfloat16
    fp8 = mybir.dt.float8e4
    Alu = mybir.AluOpType
    Act = mybir.ActivationFunctionType

    nc = bass.Bass("TRN2", debug=False, enable_asserts=False, num_devices=NCORES)
    # P pre-tiled [batch, partition, tile, d]: each partition's chunk is one
    # contiguous run in DRAM -> 128 large descriptors per transfer.
    p_ext = nc.declare_dram_parameter("p", [BPC, PT, TILES, DP], bf16, isOutput=False)
    p8t_ext = nc.declare_dram_parameter("p8t", [BPC, PT, 2, 8, 512], fp8, isOutput=False)
    invkq_ext = nc.declare_dram_parameter("invkq", [1, D], f32, isOutput=False)
    ind8_ext = nc.declare_dram_parameter("ind8", [PT, 4], fp8, isOutput=False)
    onesw_ext = nc.declare_dram_parameter("onesw", [PT, 1], f32, isOutput=False)
    out_ext = nc.declare_dram_parameter("out", [BPC, D], f32, isOutput=True)

    with tile.TileContext(nc) as tc, ExitStack() as ctx:
        consts = ctx.enter_context(tc.tile_pool(name="consts", bufs=1))
        ppool = ctx.enter_context(tc.tile_pool(name="ppool", bufs=BPC))
        p8pool = ctx.enter_context(tc.tile_pool(name="p8pool", bufs=BPC))
        erpool = ctx.enter_context(tc.tile_pool(name="erpool", bufs=2))
        ecpool = ctx.enter_context(tc.tile_pool(name="ecpool", bufs=3))
        usbp = ctx.enter_context(tc.tile_pool(name="usbp", bufs=1))
        small = ctx.enter_context(tc.tile_pool(name="small", bufs=2))
        spsum = ctx.enter_context(tc.tile_pool(name="spsum", bufs=2, space="PSUM"))
        upsum = ctx.enter_context(tc.tile_pool(name="upsum", bufs=2, space="PSUM"))
        mpsum = ctx.enter_context(tc.tile_pool(name="mpsum", bufs=1, space="PSUM"))
        ctx.enter_context(
            nc.allow_low_precision(
                reason="e stored bf16 (0.1% on softmax weights) and scores "
                "via fp8 matmuls; both far inside the 2e-2 gate"
            )
        )

        rings = [nc.sync, nc.scalar]

        invkq = consts.tile([1, D], f32, tag="invkq")
        nc.sync.dma_start(out=invkq[:, :], in_=invkq_ext[:, :])
        ind8 = consts.tile([PT, 4], fp8, tag="ind8")
        nc.scalar.dma_start(out=ind8[:, :], in_=ind8_ext[:, :])
        onesw = consts.tile([PT, 1], f32, tag="onesw")
        nc.sync.dma_start(out=onesw[:, :], in_=onesw_ext[:, :])

        # usb: merge-matmul rhs staging.  Zeroed once; per batch only
        # partitions 0-32 are rewritten, the rest stay 0 so the full-ones
        # merge weights see no garbage.
        usb = usbp.tile([PT, D + 1], f32, tag="usb")
        nc.vector.memset(usb[:, :], 0.0)

        # All bulk loads queued up-front, alternating HWDGE rings.  Per
        # batch: p8t first (score chain starts early), then 4 P chunks.
        p_tiles, p8_tiles = [], []
        dma_idx = 0

        def ring():
            nonlocal dma_idx
            eng = rings[dma_idx % 2]
            dma_idx += 1
            return eng

        for b in range(BPC):
            p8_b = p8pool.tile([PT, 2, 8, 512], fp8, tag="p8t")
            p8_tiles.append(p8_b)
            p_b = ppool.tile([PT, TILES, DP], bf16, tag="P")
            p_tiles.append(p_b)
        # p8t(0) and p8t(1) lead so the PE's score pipeline starts early and
        # batch b+1's scores are always ready before pooling(b) finishes.
        ring().dma_start(out=p8_tiles[0][:, :, :, :], in_=p8t_ext[0])
        ring().dma_start(out=p8_tiles[1][:, :, :, :], in_=p8t_ext[1])
        for b in range(BPC):
            if b >= 2:
                ring().dma_start(out=p8_tiles[b][:, :, :, :], in_=p8t_ext[b])
            pv = p_ext[b]
            for c in range(NCHUNK):
                lo, hi = c * CHUNK, (c + 1) * CHUNK
                ring().dma_start(out=p_tiles[b][:, lo:hi, :], in_=pv[:, lo:hi, :])

        def scores(b):
            """8 ones-weight matmul groups -> exp -> scatter e to columns."""
            p8_b = p8_tiles[b]
            sA = spsum.tile([4, 512], f32, tag="sA")
            sB = spsum.tile([4, 512], f32, tag="sB")
            banks = [sA, sB]
            er = erpool.tile([4, 2 * 512], bf16, tag="erows")
            for k in range(2):
                # 8 accumulating MMs: MM j carries d-block [32j, 32j+32) for
                # 4 l'-blocks stacked along K; the block-indicator lhsT
                # routes each l'-block's partial sum to its own PSUM
                # partition (0-3, contiguous, so exp is one 4-lane ACT op).
                for j in range(8):
                    nc.tensor.matmul(
                        out=banks[k][:, :],
                        lhsT=ind8[:, :],
                        rhs=p8_b[:, k, j, :],
                        start=(j == 0),
                        stop=(j == 7),
                    )
                nc.scalar.activation(
                    out=er[:, 512 * k:512 * (k + 1)], in_=banks[k][:, :],
                    func=Act.Exp, scale=1.0 / S8,
                )
            # Scatter e into pooling-weight columns: e_cols[p, t] with
            # l = t*128+p; host ordered l' = p*32+t, so dest partition p
            # reads 32 contiguous elements.  Groups q and q+4 share source
            # partition 32q (halves of er), so one 3D DMA covers both.
            ec = ecpool.tile([PT, TILES], bf16, tag="ecols")
            for m in range(4):
                nc.gpsimd.dma_start(
                    out=ec[32 * m:32 * (m + 1), :], in_=er[m:m + 1, :]
                )
            return ec

        def pooling(b, ec):
            """Column-tiled 2x pooling matmuls + merge + epilogue."""
            p_b = p_tiles[b]
            u_ps = upsum.tile([33, D + 1], f32, tag="U")
            for t in range(TILES):
                cp = 32 * (t % 2)
                nc.tensor.matmul(
                    out=u_ps[cp:cp + 1, 0:D + 1],
                    lhsT=ec[:, t:t + 1],
                    rhs=p_b[:, t, 0:D + 1],
                    start=(t < 2),
                    stop=(t >= TILES - 2),
                    tile_position=(0, cp),
                )
            # merge the two column-group accumulators: copy to SBUF (only
            # partitions 0-32 written; the rest are the memset zeros) and
            # contract with full-ones f32 weights.
            nc.vector.tensor_copy(out=usb[0:1, :], in_=u_ps[0:1, :])
            nc.vector.tensor_copy(out=usb[32:33, :], in_=u_ps[32:33, :])
            m_ps = mpsum.tile([1, D + 1], f32, tag="M")
            nc.tensor.matmul(
                out=m_ps[:, :], lhsT=onesw[:, 0:1], rhs=usb[:, :],
                start=True, stop=True,
            )
            rz = small.tile([1, 1], f32, tag="rz")
            nc.vector.reciprocal(out=rz[:, :], in_=m_ps[:, D:D + 1])
            osb = small.tile([1, D], f32, tag="osb")
            # out = (U * (1/Z)) * (1/kq), one fused VectorE op
            nc.vector.scalar_tensor_tensor(
                out=osb[:, :],
                in0=m_ps[:, 0:D],
                scalar=rz[:, :],
                in1=invkq[:, :],
                op0=Alu.mult,
                op1=Alu.mult,
            )
            nc.sync.dma_start(out=out_ext[b:b + 1, :], in_=osb[:, :])

        # software-pipelined: scores(b+1) issue before pooling(b) so the PE
        # always has score work while pooling weights are still in flight.
        ecs = [scores(0)]
        for b in range(1, BPC):
            ecs.append(scores(b))
            pooling(b - 1, ecs[b - 1])
        pooling(BPC - 1, ecs[BPC - 1])

    _legalize_waits(nc)
    _merge_sem_updates(nc)
    return nc


def kernel(Q, W, mask, kernel, bias):
    """Full unsharded inputs -> full [B, D] float32 output. W/bias are
    mathematically irrelevant (per-batch additive constant cancels in
    softmax), so they are not shipped to the device."""
    global LAST_RESULT
    import ml_dtypes
    from concourse.bass_utils import run_bass_kernel_spmd

    trace = os.environ.get("KERNEL_TRACE", "0") == "1"
    if trace:
        _install_ntff_shim()

    if "nc" not in _CACHE:
        _CACHE["nc"] = _build()
    nc = _CACHE["nc"]

    Q = np.asarray(Q, dtype=np.float32)
    mask_f = np.asarray(mask).astype(np.float32)
    kq = np.asarray(kernel, dtype=np.float32)[:D, 0]            # [256]
    inv_kq = np.where(kq == 0.0, 0.0, 1.0 / np.where(kq == 0.0, 1.0, kq))
    inv_kq = np.ascontiguousarray(inv_kq.reshape(1, D), dtype=np.float32)

    Pm = Q * kq[None, None, :] * mask_f[:, :, None]             # [B, L, D]

    P = np.empty((B, L, DP), dtype=np.float32)
    P[:, :, :D] = Pm
    P[:, :, D] = mask_f
    P[:, :, D + 1] = 0.0
    P = P.astype(ml_dtypes.bfloat16)
    # [core, batch, partition, tile, d] with l = tile*128 + partition
    ps = P.reshape(NCORES, BPC, TILES, PT, DP).transpose(0, 1, 3, 2, 4)

    # P8T blocks for the M=4 indicator score matmuls:
    # X[b, k, j, p, n] = S8*Pm[b, l, d] with d = 32j + p%32 and
    # l' = k*2048 + (p//32)*512 + n, l = (l'%32)*128 + l'//32  (l' = p*32+t)
    A2 = Pm * S8
    kk = np.arange(2)[None, :, None]
    pp = np.arange(PT)[:, None, None]
    nn = np.arange(512)[None, None, :]
    lprime = (pp // 32) * 1024 + kk * 512 + nn                  # [128,2,512]
    lmap = (lprime % TILES) * PT + lprime // TILES
    dmap = (32 * np.arange(8)[:, None] + np.arange(PT)[None, :] % 32)  # [8,128]
    # X[b, p, k, j, n] = S8*Pm[b, l(lprime[p,k,n]), dmap[j,p]]
    X = A2[:, lmap[:, :, None, :], dmap.T[:, None, :, None]]    # [B,128,2,8,512]
    p8s = X.astype(ml_dtypes.float8_e4m3).reshape(NCORES, BPC, PT, 2, 8, 512)

    ind8 = np.zeros((PT, 4), dtype=ml_dtypes.float8_e4m3)
    for m in range(4):
        ind8[32 * m:32 * (m + 1), m] = 1.0
    onesw = np.ones((PT, 1), dtype=np.float32)

    in_maps = []
    for i in range(NCORES):
        in_maps.append(
            {
                "p": np.ascontiguousarray(ps[i]),
                "p8t": np.ascontiguousarray(p8s[i]),
                "invkq": inv_kq,
                "ind8": ind8,
                "onesw": onesw,
            }
        )

    res = run_bass_kernel_spmd(
        nc,
        in_maps,
        core_ids=list(range(NCORES)),
        trace=trace,
        tmpdir=os.environ.get("KERNEL_TRACE_DIR") or None,
    )
    LAST_RESULT = res
    out = np.concatenate([res.results[i]["out"] for i in range(NCORES)], axis=0)
    return out.astype(np.float32)


# revision 17
# speedup vs baseline: 1.2777x; 1.1804x over previous
"""Trainium2 Bass kernel for masked softmax attention-pooling.

Reference computation (per batch b):
    scores[l] = Q[b,l,:] . kernel[:D,0]  (+ const_b, which cancels in softmax)
    alpha     = softmax_l(scores masked by mask[b])
    out[b,:]  = sum_l alpha[l] * Q[b,l,:]

Distribution: pure data parallel, 4 batches per core across 8 NeuronCores.

Design: BOTH reductions run on the TensorEngine, which streams operands at
128 elem/cycle -- the DVE/ScalarE row-sum path (tensor_reduce is hard-capped
at 1 elem/lane/cycle @0.96 GHz) cannot keep up with the DMA stream and was
the bottleneck of all earlier variants.  Two shipments of the data:

  - P   [l-partitions, d-free] bf16: pooling operand (exactly-precise path).
    Rows pre-scaled by kq (undone by a 1/kq epilogue multiply), masked rows
    zeroed, mask column appended (doubles as the Z accumulator in the
    pooling matmul: masked rows contribute exp(0)*0 = 0 to U and Z).
  - P8T [d-partitions, l-free] fp8e4m3 (x64 scale): score operand.
    scores come from ones-weight matmuls contracting d over the partitions;
    fp8 keeps this copy at half weight (score quantization noise ~1.4e-2,
    inside the 2e-2 gate with the bf16-exact pooling path).

Score path per batch: 8 matmul groups (512 l-columns each, 2 d-halves
accumulated) write S*s into PSUM spread over partitions {0,32,64,96} x 2
banks via column tile_position; ScalarE exp(s/S) reads the 4 partitions
strided and a 4-descriptor SBUF->SBUF scatter DMA per group-pair transposes
e from [4 rows, 1024] into the [128, 32] column layout the pooling matmuls
need as stationary weights (the host orders P8T columns l' = p*32 + t to
make every scatter run contiguous).  Pooling matmuls are column-tiled 2x
(tiles alternate PE column groups 0/32) and merged by a tiny ones-weight
f32 matmul per batch; epilogue out = U * (1/Z) * (1/kq) in one fused
VectorE op.  DMA: all bulk loads queued up-front alternating the two HWDGE
rings (~210 GB/s each); per-batch order p8t then 4 P chunks, so each
batch's score chain completes while its pooling data streams in.
"""

import os

import numpy as np

B, L, D = 32, 4096, 256
DP = D + 2                 # +mask column (doubles as Z accumulator), +pad
NCORES = 8
BPC = B // NCORES          # batches per core
PT = 128                   # partition tile (l rows per tile)
TILES = L // PT            # 32 l-tiles per batch
CHUNK = 8                  # l-tiles per pooling DMA/matmul group
NCHUNK = TILES // CHUNK
S8 = 64.0                  # fp8 pre-scale: |Q*kq| <~ 1.2 -> |P8T| <~ 80 < 448,
                           # keeps small values out of the subnormal floor
NG = 8                     # score matmul groups per batch (512 l' each)

_CACHE = {}
LAST_RESULT = None


def _install_ntff_shim():
    """Register the missing antenv.axon_hooks module so trace=True works."""
    import sys
    import types

    if "antenv.axon_hooks" in sys.modules:
        return
    mod = types.ModuleType("antenv.axon_hooks")
    state = {"hook": None}

    def set_axon_ntff_profile_hook(h):
        state["hook"] = h

    def get_axon_ntff_profile_hook():
        return state["hook"]

    mod.set_axon_ntff_profile_hook = set_axon_ntff_profile_hook
    mod.get_axon_ntff_profile_hook = get_axon_ntff_profile_hook
    sys.modules["antenv.axon_hooks"] = mod
    try:
        import antenv

        antenv.axon_hooks = mod
        from trn_agent_boot.trn_boot import _ntff_profile_via_ctypes

        set_axon_ntff_profile_hook(_ntff_profile_via_ctypes("/opt/axon/libaxon_pjrt.so"))
    except Exception:
        pass


def _legalize_waits(nc):
    """This walrus build accepts at most one sync wait per instruction.
    Tile emits several on some instructions; move the extras onto injected
    NOPs on the same engine immediately before the instruction (engine
    streams execute in block order, so the waits still happen-before)."""
    from concourse import mybir

    counter = [0]
    for fn in nc.m.functions:
        for bb in fn.blocks:
            insts = bb.instructions
            i = 0
            while i < len(insts):
                inst = insts[i]
                si = inst.sync_info
                waits = list(si.on_wait) if si and si.on_wait else []
                if len(waits) > 1:
                    si.on_wait = [waits[0]]
                    for w in waits[1:]:
                        counter[0] += 1
                        nop = mybir.InstNoOp(
                            name=f"legalize-wait-{counter[0]}", ins=[], outs=[]
                        )
                        nop.engine = inst.engine
                        nop.sync_info = mybir.SyncInfo(on_wait=[w], on_update=[])
                        insts.insert(i, nop)
                        i += 1
                i += 1


def _merge_sem_updates(nc):
    """Each instruction-attached sem increment lowers to a serialized EVT_SEM
    write on the issuing engine (~50-115 ns); with 200+ matmuls the PE pays
    several us for these at the kernel tail. walrus requires UpdateValue == 1,
    so instead of merging values we DROP every increment whose running count
    is never awaited and rebase all wait thresholds to their rank among the
    kept increments -- the waiter still unblocks on completion of exactly the
    same producer instruction."""
    from concourse import mybir

    skip_types = ("InstDMACopy", "InstEventSemaphore", "InstDrain", "InstISA")
    blocks = [bb for fn in nc.m.functions for bb in fn.blocks]

    awaited = {}
    sem_info = {}
    for bb in blocks:
        for inst in bb.instructions:
            si = inst.sync_info
            if si is None:
                continue
            for w in si.on_wait or []:
                if (
                    w.sync_type != "semaphore"
                    or w.wait_mode != "sem-ge-imm"
                    or w.wait_reg is not None
                ):
                    sem_info[w.id] = None  # unknown semantics; leave alone
                    continue
                awaited.setdefault(w.id, set()).add(w.wait_value)
            for u in si.on_update or []:
                if u.sync_type != "semaphore":
                    continue
                info = sem_info.setdefault(u.id, {"engine": inst.engine, "ok": True})
                if info is None:
                    continue
                if (
                    u.update_mode != "sem-inc"
                    or u.update_value != 1
                    or u.update_reg is not None
                    or inst.engine != info["engine"]
                    or type(inst).__name__ in skip_types
                ):
                    info["ok"] = False

    mergeable = {
        sid
        for sid, info in sem_info.items()
        if info is not None and info["ok"] and awaited.get(sid)
    }

    for sid in mergeable:
        targets = awaited[sid]
        rank = {v: i + 1 for i, v in enumerate(sorted(targets))}
        cum = 0
        for bb in blocks:
            for inst in bb.instructions:
                si = inst.sync_info
                if si is None:
                    continue
                if si.on_update:
                    ups = list(si.on_update)
                    changed = False
                    for u in list(ups):
                        if u.sync_type != "semaphore" or u.id != sid:
                            continue
                        cum += 1
                        if cum not in targets:
                            ups = [x for x in ups if x is not u]
                            changed = True
                    if changed:
                        si.on_update = ups
                if si.on_wait:
                    ws = list(si.on_wait)
                    changed = False
                    for i, w in enumerate(ws):
                        if w.sync_type == "semaphore" and w.id == sid:
                            ws[i] = mybir.SyncWait(
                                sync_type="semaphore",
                                id=sid,
                                ant_name=w.ant_name,
                                wait_mode="sem-ge-imm",
                                wait_value=rank[w.wait_value],
                            )
                            changed = True
                    if changed:
                        si.on_wait = ws


def _build():
    from contextlib import ExitStack

    from concourse import bass, mybir, tile

    f32 = mybir.dt.float32
    bf16 = mybir.dt.bfloat16
    fp8 = mybir.dt.float8e4
    Alu = mybir.AluOpType
    Act = mybir.ActivationFunctionType

    nc = bass.Bass("TRN2", debug=False, enable_asserts=False, num_devices=NCORES)
    # P pre-tiled [batch, partition, tile, d]: each partition's chunk is one
    # contiguous run in DRAM -> 128 large descriptors per transfer.
    p_ext = nc.declare_dram_parameter("p", [BPC, PT, TILES, DP], bf16, isOutput=False)
    p8t_ext = nc.declare_dram_parameter("p8t", [BPC, PT, 2, 8, 512], fp8, isOutput=False)
    invkq_ext = nc.declare_dram_parameter("invkq", [1, D], f32, isOutput=False)
    ind8_ext = nc.declare_dram_parameter("ind8", [PT, 4], fp8, isOutput=False)
    onesw_ext = nc.declare_dram_parameter("onesw", [PT, 1], f32, isOutput=False)
    out_ext = nc.declare_dram_parameter("out", [BPC, D], f32, isOutput=True)

    with tile.TileContext(nc) as tc, ExitStack() as ctx:
        consts = ctx.enter_context(tc.tile_pool(name="consts", bufs=1))
        ppool = ctx.enter_context(tc.tile_pool(name="ppool", bufs=BPC))
        p8pool = ctx.enter_context(tc.tile_pool(name="p8pool", bufs=BPC))
        erpool = ctx.enter_context(tc.tile_pool(name="erpool", bufs=2))
        ecpool = ctx.enter_context(tc.tile_pool(name="ecpool", bufs=3))
        usbp = ctx.enter_context(tc.tile_pool(name="usbp", bufs=1))
        small = ctx.enter_context(tc.tile_pool(name="small", bufs=2))
        spsum = ctx.enter_context(tc.tile_pool(name="spsum", bufs=2, space="PSUM"))
        upsum = ctx.enter_context(tc.tile_pool(name="upsum", bufs=2, space="PSUM"))
        mpsum = ctx.enter_context(tc.tile_pool(name="mpsum", bufs=1, space="PSUM"))
        ctx.enter_context(
            nc.allow_low_precision(
                reason="e stored bf16 (0.1% on softmax weights) and scores "
                "via fp8 matmuls; both far inside the 2e-2 gate"
            )
        )

        rings = [nc.sync, nc.scalar]

        invkq = consts.tile([1, D], f32, tag="invkq")
        nc.sync.dma_start(out=invkq[:, :], in_=invkq_ext[:, :])
        ind8 = consts.tile([PT, 4], fp8, tag="ind8")
        nc.scalar.dma_start(out=ind8[:, :], in_=ind8_ext[:, :])
        onesw = consts.tile([PT, 1], f32, tag="onesw")
        nc.sync.dma_start(out=onesw[:, :], in_=onesw_ext[:, :])

        # usb: merge-matmul rhs staging.  Zeroed once; per batch only
        # partitions 0-32 are rewritten, the rest stay 0 so the full-ones
        # merge weights see no garbage.
        usb = usbp.tile([PT, D + 1], f32, tag="usb")
        nc.vector.memset(usb[:, :], 0.0)

        # Bulk loads are emitted INTERLEAVED with the compute ops (see the
        # pipeline at the bottom): a dma_start whose DMAHW flow-control lane
        # is still busy stalls the issuing engine, and anything queued
        # behind it (exp!) head-of-line blocks.  Each p8t ships as two
        # ring-parallel halves (the k dim) and each P batch as two 16-tile
        # halves, so both rings stay fed with ~0.5-1 MB transfers.
        p_tiles, p8_tiles = [], []
        for b in range(BPC):
            p8_b = p8pool.tile([PT, 2, 8, 512], fp8, tag="p8t")
            p8_tiles.append(p8_b)
            p_b = ppool.tile([PT, TILES, DP], bf16, tag="P")
            p_tiles.append(p_b)

        def load_p8t(b):
            for k in range(2):
                rings[k].dma_start(
                    out=p8_tiles[b][:, k, :, :], in_=p8t_ext[b, :, k]
                )

        def load_p(b):
            for h in range(2):
                lo, hi = 16 * h, 16 * (h + 1)
                rings[h].dma_start(
                    out=p_tiles[b][:, lo:hi, :], in_=p_ext[b][:, lo:hi, :]
                )

        def scores(b):
            """8 ones-weight matmul groups -> exp -> scatter e to columns."""
            p8_b = p8_tiles[b]
            sA = spsum.tile([4, 512], f32, tag="sA")
            sB = spsum.tile([4, 512], f32, tag="sB")
            banks = [sA, sB]
            er = erpool.tile([4, 2 * 512], bf16, tag="erows")
            for k in range(2):
                # 8 accumulating MMs: MM j carries d-block [32j, 32j+32) for
                # 4 l'-blocks stacked along K; the block-indicator lhsT
                # routes each l'-block's partial sum to its own PSUM
                # partition (0-3, contiguous, so exp is one 4-lane ACT op).
                for j in range(8):
                    nc.tensor.matmul(
                        out=banks[k][:, :],
                        lhsT=ind8[:, :],
                        rhs=p8_b[:, k, j, :],
                        start=(j == 0),
                        stop=(j == 7),
                    )
                nc.scalar.activation(
                    out=er[:, 512 * k:512 * (k + 1)], in_=banks[k][:, :],
                    func=Act.Exp, scale=1.0 / S8,
                )
            # Scatter e into pooling-weight columns: e_cols[p, t] with
            # l = t*128+p; host ordered l' = p*32+t, so dest partition p
            # reads 32 contiguous elements.  Groups q and q+4 share source
            # partition 32q (halves of er), so one 3D DMA covers both.
            ec = ecpool.tile([PT, TILES], bf16, tag="ecols")
            for m in range(4):
                nc.gpsimd.dma_start(
                    out=ec[32 * m:32 * (m + 1), :], in_=er[m:m + 1, :]
                )
            return ec

        def pooling(b, ec):
            """Column-tiled 2x pooling matmuls + merge + epilogue."""
            p_b = p_tiles[b]
            u_ps = upsum.tile([33, D + 1], f32, tag="U")
            for t in range(TILES):
                cp = 32 * (t % 2)
                nc.tensor.matmul(
                    out=u_ps[cp:cp + 1, 0:D + 1],
                    lhsT=ec[:, t:t + 1],
                    rhs=p_b[:, t, 0:D + 1],
                    start=(t < 2),
                    stop=(t >= TILES - 2),
                    tile_position=(0, cp),
                )
            # merge the two column-group accumulators: copy to SBUF (only
            # partitions 0-32 written; the rest are the memset zeros) and
            # contract with full-ones f32 weights.
            nc.vector.tensor_copy(out=usb[0:1, :], in_=u_ps[0:1, :])
            nc.vector.tensor_copy(out=usb[32:33, :], in_=u_ps[32:33, :])
            m_ps = mpsum.tile([1, D + 1], f32, tag="M")
            nc.tensor.matmul(
                out=m_ps[:, :], lhsT=onesw[:, 0:1], rhs=usb[:, :],
                start=True, stop=True,
            )
            rz = small.tile([1, 1], f32, tag="rz")
            nc.vector.reciprocal(out=rz[:, :], in_=m_ps[:, D:D + 1])
            osb = small.tile([1, D], f32, tag="osb")
            # out = (U * (1/Z)) * (1/kq), one fused VectorE op
            nc.vector.scalar_tensor_tensor(
                out=osb[:, :],
                in0=m_ps[:, 0:D],
                scalar=rz[:, :],
                in1=invkq[:, :],
                op0=Alu.mult,
                op1=Alu.mult,
            )
            nc.scalar.dma_start(out=out_ext[b:b + 1, :], in_=osb[:, :])

        # software-pipelined; load issues interleave with compute emission
        # so no latency-critical op queues behind a stalled dma_start.
        load_p8t(0)
        load_p8t(1)
        load_p(0)
        ecs = [scores(0)]
        load_p8t(2)
        load_p(1)
        ecs.append(scores(1))
        pooling(0, ecs[0])
        load_p8t(3)
        load_p(2)
        ecs.append(scores(2))
        pooling(1, ecs[1])
        load_p(3)
        ecs.append(scores(3))
        pooling(2, ecs[2])
        pooling(3, ecs[3])

    _legalize_waits(nc)
    _merge_sem_updates(nc)
    return nc


def kernel(Q, W, mask, kernel, bias):
    """Full unsharded inputs -> full [B, D] float32 output. W/bias are
    mathematically irrelevant (per-batch additive constant cancels in
    softmax), so they are not shipped to the device."""
    global LAST_RESULT
    import ml_dtypes
    from concourse.bass_utils import run_bass_kernel_spmd

    trace = os.environ.get("KERNEL_TRACE", "0") == "1"
    if trace:
        _install_ntff_shim()

    if "nc" not in _CACHE:
        _CACHE["nc"] = _build()
    nc = _CACHE["nc"]

    Q = np.asarray(Q, dtype=np.float32)
    mask_f = np.asarray(mask).astype(np.float32)
    kq = np.asarray(kernel, dtype=np.float32)[:D, 0]            # [256]
    inv_kq = np.where(kq == 0.0, 0.0, 1.0 / np.where(kq == 0.0, 1.0, kq))
    inv_kq = np.ascontiguousarray(inv_kq.reshape(1, D), dtype=np.float32)

    Pm = Q * kq[None, None, :] * mask_f[:, :, None]             # [B, L, D]

    P = np.empty((B, L, DP), dtype=np.float32)
    P[:, :, :D] = Pm
    P[:, :, D] = mask_f
    P[:, :, D + 1] = 0.0
    P = P.astype(ml_dtypes.bfloat16)
    # [core, batch, partition, tile, d] with l = tile*128 + partition
    ps = P.reshape(NCORES, BPC, TILES, PT, DP).transpose(0, 1, 3, 2, 4)

    # P8T blocks for the M=4 indicator score matmuls:
    # X[b, k, j, p, n] = S8*Pm[b, l, d] with d = 32j + p%32 and
    # l' = k*2048 + (p//32)*512 + n, l = (l'%32)*128 + l'//32  (l' = p*32+t)
    A2 = Pm * S8
    kk = np.arange(2)[None, :, None]
    pp = np.arange(PT)[:, None, None]
    nn = np.arange(512)[None, None, :]
    lprime = (pp // 32) * 1024 + kk * 512 + nn                  # [128,2,512]
    lmap = (lprime % TILES) * PT + lprime // TILES
    dmap = (32 * np.arange(8)[:, None] + np.arange(PT)[None, :] % 32)  # [8,128]
    # X[b, p, k, j, n] = S8*Pm[b, l(lprime[p,k,n]), dmap[j,p]]
    X = A2[:, lmap[:, :, None, :], dmap.T[:, None, :, None]]    # [B,128,2,8,512]
    p8s = X.astype(ml_dtypes.float8_e4m3).reshape(NCORES, BPC, PT, 2, 8, 512)

    ind8 = np.zeros((PT, 4), dtype=ml_dtypes.float8_e4m3)
    for m in range(4):
        ind8[32 * m:32 * (m + 1), m] = 1.0
    onesw = np.ones((PT, 1), dtype=np.float32)

    in_maps = []
    for i in range(NCORES):
        in_maps.append(
            {
                "p": np.ascontiguousarray(ps[i]),
                "p8t": np.ascontiguousarray(p8s[i]),
                "invkq": inv_kq,
                "ind8": ind8,
                "onesw": onesw,
            }
        )

    res = run_bass_kernel_spmd(
        nc,
        in_maps,
        core_ids=list(range(NCORES)),
        trace=trace,
        tmpdir=os.environ.get("KERNEL_TRACE_DIR") or None,
    )
    LAST_RESULT = res
    out = np.concatenate([res.results[i]["out"] for i in range(NCORES)], axis=0)
    return out.astype(np.float32)


# revision 20
# speedup vs baseline: 1.3827x; 1.0822x over previous
"""Trainium2 Bass kernel for masked softmax attention-pooling.

Reference computation (per batch b):
    scores[l] = Q[b,l,:] . kernel[:D,0]  (+ const_b, which cancels in softmax)
    alpha     = softmax_l(scores masked by mask[b])
    out[b,:]  = sum_l alpha[l] * Q[b,l,:]

Distribution: pure data parallel, 4 batches per core across 8 NeuronCores.

Design: BOTH reductions run on the TensorEngine, which streams operands at
128 elem/cycle -- the DVE/ScalarE row-sum path (tensor_reduce is hard-capped
at 1 elem/lane/cycle @0.96 GHz) cannot keep up with the DMA stream and was
the bottleneck of all earlier variants.  Two shipments of the data:

  - P   [l-partitions, d-free] bf16: pooling operand (exactly-precise path).
    Rows pre-scaled by kq (undone by a 1/kq epilogue multiply), masked rows
    zeroed, mask column appended (doubles as the Z accumulator in the
    pooling matmul: masked rows contribute exp(0)*0 = 0 to U and Z).
  - P8T [d-partitions, l-free] fp8e4m3 (x64 scale): score operand.
    scores come from ones-weight matmuls contracting d over the partitions;
    fp8 keeps this copy at half weight (score quantization noise ~1.4e-2,
    inside the 2e-2 gate with the bf16-exact pooling path).

Score path per batch: 8 matmul groups (512 l-columns each, 2 d-halves
accumulated) write S*s into PSUM spread over partitions {0,32,64,96} x 2
banks via column tile_position; ScalarE exp(s/S) reads the 4 partitions
strided and a 4-descriptor SBUF->SBUF scatter DMA per group-pair transposes
e from [4 rows, 1024] into the [128, 32] column layout the pooling matmuls
need as stationary weights (the host orders P8T columns l' = p*32 + t to
make every scatter run contiguous).  Pooling matmuls are column-tiled 2x
(tiles alternate PE column groups 0/32) and merged by a tiny ones-weight
f32 matmul per batch; epilogue out = U * (1/Z) * (1/kq) in one fused
VectorE op.  DMA: all bulk loads queued up-front alternating the two HWDGE
rings (~210 GB/s each); per-batch order p8t then 4 P chunks, so each
batch's score chain completes while its pooling data streams in.
"""

import os

import numpy as np

B, L, D = 32, 4096, 256
DP = D + 2                 # +mask column (doubles as Z accumulator), +pad
NCORES = 8
BPC = B // NCORES          # batches per core
PT = 128                   # partition tile (l rows per tile)
TILES = L // PT            # 32 l-tiles per batch
CHUNK = 8                  # l-tiles per pooling DMA/matmul group
NCHUNK = TILES // CHUNK
S8 = 256.0                 # fp8 pre-scale for the low-|kq| column block:
                           # |Q*kq_lo| <~ 0.25 -> |P8T| <~ 64 < 448, keeps
                           # small values out of the subnormal floor
DA = 128                   # columns scored on DVE (top |kq|, from bf16 P)
DB = D - DA                # columns scored on PE (bottom |kq|, fp8 P8T)

_CACHE = {}
LAST_RESULT = None


def _install_ntff_shim():
    """Register the missing antenv.axon_hooks module so trace=True works."""
    import sys
    import types

    if "antenv.axon_hooks" in sys.modules:
        return
    mod = types.ModuleType("antenv.axon_hooks")
    state = {"hook": None}

    def set_axon_ntff_profile_hook(h):
        state["hook"] = h

    def get_axon_ntff_profile_hook():
        return state["hook"]

    mod.set_axon_ntff_profile_hook = set_axon_ntff_profile_hook
    mod.get_axon_ntff_profile_hook = get_axon_ntff_profile_hook
    sys.modules["antenv.axon_hooks"] = mod
    try:
        import antenv

        antenv.axon_hooks = mod
        from trn_agent_boot.trn_boot import _ntff_profile_via_ctypes

        set_axon_ntff_profile_hook(_ntff_profile_via_ctypes("/opt/axon/libaxon_pjrt.so"))
    except Exception:
        pass


def _legalize_waits(nc):
    """This walrus build accepts at most one sync wait per instruction.
    Tile emits several on some instructions; move the extras onto injected
    NOPs on the same engine immediately before the instruction (engine
    streams execute in block order, so the waits still happen-before)."""
    from concourse import mybir

    counter = [0]
    for fn in nc.m.functions:
        for bb in fn.blocks:
            insts = bb.instructions
            i = 0
            while i < len(insts):
                inst = insts[i]
                si = inst.sync_info
                waits = list(si.on_wait) if si and si.on_wait else []
                if len(waits) > 1:
                    si.on_wait = [waits[0]]
                    for w in waits[1:]:
                        counter[0] += 1
                        nop = mybir.InstNoOp(
                            name=f"legalize-wait-{counter[0]}", ins=[], outs=[]
                        )
                        nop.engine = inst.engine
                        nop.sync_info = mybir.SyncInfo(on_wait=[w], on_update=[])
                        insts.insert(i, nop)
                        i += 1
                i += 1


def _merge_sem_updates(nc):
    """Each instruction-attached sem increment lowers to a serialized EVT_SEM
    write on the issuing engine (~50-115 ns); with 200+ matmuls the PE pays
    several us for these at the kernel tail. walrus requires UpdateValue == 1,
    so instead of merging values we DROP every increment whose running count
    is never awaited and rebase all wait thresholds to their rank among the
    kept increments -- the waiter still unblocks on completion of exactly the
    same producer instruction."""
    from concourse import mybir

    skip_types = ("InstDMACopy", "InstEventSemaphore", "InstDrain", "InstISA")
    blocks = [bb for fn in nc.m.functions for bb in fn.blocks]

    awaited = {}
    sem_info = {}
    for bb in blocks:
        for inst in bb.instructions:
            si = inst.sync_info
            if si is None:
                continue
            for w in si.on_wait or []:
                if (
                    w.sync_type != "semaphore"
                    or w.wait_mode != "sem-ge-imm"
                    or w.wait_reg is not None
                ):
                    sem_info[w.id] = None  # unknown semantics; leave alone
                    continue
                awaited.setdefault(w.id, set()).add(w.wait_value)
            for u in si.on_update or []:
                if u.sync_type != "semaphore":
                    continue
                info = sem_info.setdefault(u.id, {"engine": inst.engine, "ok": True})
                if info is None:
                    continue
                if (
                    u.update_mode != "sem-inc"
                    or u.update_value != 1
                    or u.update_reg is not None
                    or inst.engine != info["engine"]
                    or type(inst).__name__ in skip_types
                ):
                    info["ok"] = False

    mergeable = {
        sid
        for sid, info in sem_info.items()
        if info is not None and info["ok"] and awaited.get(sid)
    }

    for sid in mergeable:
        targets = awaited[sid]
        rank = {v: i + 1 for i, v in enumerate(sorted(targets))}
        cum = 0
        for bb in blocks:
            for inst in bb.instructions:
                si = inst.sync_info
                if si is None:
                    continue
                if si.on_update:
                    ups = list(si.on_update)
                    changed = False
                    for u in list(ups):
                        if u.sync_type != "semaphore" or u.id != sid:
                            continue
                        cum += 1
                        if cum not in targets:
                            ups = [x for x in ups if x is not u]
                            changed = True
                    if changed:
                        si.on_update = ups
                if si.on_wait:
                    ws = list(si.on_wait)
                    changed = False
                    for i, w in enumerate(ws):
                        if w.sync_type == "semaphore" and w.id == sid:
                            ws[i] = mybir.SyncWait(
                                sync_type="semaphore",
                                id=sid,
                                ant_name=w.ant_name,
                                wait_mode="sem-ge-imm",
                                wait_value=rank[w.wait_value],
                            )
                            changed = True
                    if changed:
                        si.on_wait = ws


def _build():
    from contextlib import ExitStack

    from concourse import bass, mybir, tile

    f32 = mybir.dt.float32
    bf16 = mybir.dt.bfloat16
    fp8 = mybir.dt.float8e4
    Alu = mybir.AluOpType
    Act = mybir.ActivationFunctionType

    nc = bass.Bass("TRN2", debug=False, enable_asserts=False, num_devices=NCORES)
    # P pre-tiled [batch, partition, tile, d]: each partition's chunk is one
    # contiguous run in DRAM -> 128 large descriptors per transfer.
    p_ext = nc.declare_dram_parameter("p", [BPC, PT, TILES, DP], bf16, isOutput=False)
    p8t_ext = nc.declare_dram_parameter("p8t", [BPC, PT, 2, 4, 512], fp8, isOutput=False)
    invkq_ext = nc.declare_dram_parameter("invkq", [1, D], f32, isOutput=False)
    ind8_ext = nc.declare_dram_parameter("ind8", [PT, 4], fp8, isOutput=False)
    onesw_ext = nc.declare_dram_parameter("onesw", [PT, 1], f32, isOutput=False)
    out_ext = nc.declare_dram_parameter("out", [BPC, D], f32, isOutput=True)

    with tile.TileContext(nc) as tc, ExitStack() as ctx:
        consts = ctx.enter_context(tc.tile_pool(name="consts", bufs=1))
        ppool = ctx.enter_context(tc.tile_pool(name="ppool", bufs=BPC))
        p8pool = ctx.enter_context(tc.tile_pool(name="p8pool", bufs=BPC))
        erpool = ctx.enter_context(tc.tile_pool(name="erpool", bufs=2))
        ecpool = ctx.enter_context(tc.tile_pool(name="ecpool", bufs=8))
        hpool = ctx.enter_context(tc.tile_pool(name="hpool", bufs=2))
        spool = ctx.enter_context(tc.tile_pool(name="spool", bufs=4))
        usbp = ctx.enter_context(tc.tile_pool(name="usbp", bufs=1))
        small = ctx.enter_context(tc.tile_pool(name="small", bufs=2))
        spsum = ctx.enter_context(tc.tile_pool(name="spsum", bufs=2, space="PSUM"))
        upsum = ctx.enter_context(tc.tile_pool(name="upsum", bufs=2, space="PSUM"))
        mpsum = ctx.enter_context(tc.tile_pool(name="mpsum", bufs=1, space="PSUM"))
        wpsum = ctx.enter_context(tc.tile_pool(name="wpsum", bufs=1, space="PSUM"))
        ctx.enter_context(
            nc.allow_low_precision(
                reason="e stored bf16 (0.1% on softmax weights) and scores "
                "via fp8 matmuls; both far inside the 2e-2 gate"
            )
        )

        rings = [nc.sync, nc.scalar]

        invkq = consts.tile([1, D], f32, tag="invkq")
        nc.sync.dma_start(out=invkq[:, :], in_=invkq_ext[:, :])
        ind8 = consts.tile([PT, 4], fp8, tag="ind8")
        nc.scalar.dma_start(out=ind8[:, :], in_=ind8_ext[:, :])
        onesw = consts.tile([PT, 1], f32, tag="onesw")
        nc.sync.dma_start(out=onesw[:, :], in_=onesw_ext[:, :])

        # usb: merge-matmul rhs staging.  Zeroed once; per batch only
        # partitions 0-32 are rewritten, the rest stay 0 so the full-ones
        # merge weights see no garbage.
        usb = usbp.tile([PT, D + 1], f32, tag="usb")
        nc.vector.memset(usb[:, :], 0.0)

        # Bulk loads are emitted INTERLEAVED with the compute ops (see the
        # pipeline at the bottom): a dma_start whose DMAHW flow-control lane
        # is still busy stalls the issuing engine, and anything queued
        # behind it (exp!) head-of-line blocks.  Each p8t ships as two
        # ring-parallel halves (the k dim) and each P batch as two 16-tile
        # halves, so both rings stay fed with ~0.5-1 MB transfers.
        p_tiles, p8_tiles = [], []
        for b in range(BPC):
            p8_b = p8pool.tile([PT, 2, 4, 512], fp8, tag="p8t")
            p8_tiles.append(p8_b)
            p_b = ppool.tile([PT, TILES, DP], bf16, tag="P")
            p_tiles.append(p_b)

        def load_p8t(b):
            for k in range(2):
                rings[k].dma_start(
                    out=p8_tiles[b][:, k, :, :], in_=p8t_ext[b, :, k]
                )

        def load_p(b):
            for h in range(2):
                lo, hi = 16 * h, 16 * (h + 1)
                rings[h].dma_start(
                    out=p_tiles[b][:, lo:hi, :], in_=p_ext[b][:, lo:hi, :]
                )

        def scores_b_part(b):
            """Low-|kq| column block: indicator matmuls -> s rows (bf16,
            descaled by 1/S8 in the ACT copy) -> scatter to column layout."""
            p8_b = p8_tiles[b]
            sA = spsum.tile([4, 512], f32, tag="sA")
            sB = spsum.tile([4, 512], f32, tag="sB")
            banks = [sA, sB]
            er = erpool.tile([4, 2 * 512], bf16, tag="erows")
            for k in range(2):
                for j in range(4):
                    nc.tensor.matmul(
                        out=banks[k][:, :],
                        lhsT=ind8[:, :],
                        rhs=p8_b[:, k, j, :],
                        start=(j == 0),
                        stop=(j == 3),
                    )
                nc.scalar.activation(
                    out=er[:, 512 * k:512 * (k + 1)], in_=banks[k][:, :],
                    func=Act.Copy, scale=1.0 / S8,
                )
            sbc = ecpool.tile([PT, TILES], bf16, tag="sbc")
            for m in range(4):
                nc.gpsimd.dma_start(
                    out=sbc[32 * m:32 * (m + 1), :], in_=er[m:m + 1, :]
                )
            return sbc

        def scores_a_half(b, h, sbc, ec):
            """Top-|kq| block scored on DVE straight from bf16 P (columns
            0:DA of each row, split in two packed-add halves), fused with
            the scattered B-part and exp'd into pooling-weight columns."""
            lo, hi = 16 * h, 16 * (h + 1)
            p_b = p_tiles[b]
            ha = hpool.tile([PT, 16, DA // 2], bf16, tag="ha")
            nc.vector.tensor_tensor(
                out=ha[:, :, :],
                in0=p_b[:, lo:hi, 0:DA // 2],
                in1=p_b[:, lo:hi, DA // 2:DA],
                op=Alu.add,
            )
            sa = spool.tile([PT, 16], bf16, tag="sa")
            nc.vector.tensor_reduce(
                out=sa[:, :], in_=ha[:, :, :],
                axis=mybir.AxisListType.X, op=Alu.add,
            )
            st = spool.tile([PT, 16], bf16, tag="st")
            nc.vector.tensor_tensor(
                out=st[:, :], in0=sa[:, :], in1=sbc[:, lo:hi], op=Alu.add,
            )
            nc.scalar.activation(
                out=ec[:, lo:hi], in_=st[:, :], func=Act.Exp
            )

        def pooling(b, ec):
            """Column-tiled 2x pooling matmuls + merge + epilogue."""
            p_b = p_tiles[b]
            u_ps = upsum.tile([33, D + 1], f32, tag="U")
            for t in range(TILES):
                cp = 32 * (t % 2)
                nc.tensor.matmul(
                    out=u_ps[cp:cp + 1, 0:D + 1],
                    lhsT=ec[:, t:t + 1],
                    rhs=p_b[:, t, 0:D + 1],
                    start=(t < 2),
                    stop=(t >= TILES - 2),
                    tile_position=(0, cp),
                )
            # merge the two column-group accumulators: copy to SBUF (only
            # partitions 0-32 written; the rest are the memset zeros) and
            # contract with full-ones f32 weights.
            nc.vector.tensor_copy(out=usb[0:1, :], in_=u_ps[0:1, :])
            nc.vector.tensor_copy(out=usb[32:33, :], in_=u_ps[32:33, :])
            m_ps = mpsum.tile([1, D + 1], f32, tag="M")
            nc.tensor.matmul(
                out=m_ps[:, :], lhsT=onesw[:, 0:1], rhs=usb[:, :],
                start=True, stop=True,
            )
            rz = small.tile([1, 1], f32, tag="rz")
            nc.vector.reciprocal(out=rz[:, :], in_=m_ps[:, D:D + 1])
            osb = small.tile([1, D], f32, tag="osb")
            # out = (U * (1/Z)) * (1/kq), one fused VectorE op
            nc.vector.scalar_tensor_tensor(
                out=osb[:, :],
                in0=m_ps[:, 0:D],
                scalar=rz[:, :],
                in1=invkq[:, :],
                op0=Alu.mult,
                op1=Alu.mult,
            )
            nc.scalar.dma_start(out=out_ext[b:b + 1, :], in_=osb[:, :])

        # PE warm-up: a few throwaway matmuls over already-resident zeros
        # keep the HAM activity window busy while the first loads stream,
        # so the real score/pooling matmuls run at 2.4 GHz, not 1.2.
        warm = wpsum.tile([1, D + 1], f32, tag="warm")
        for _ in range(10):
            nc.tensor.matmul(
                out=warm[:, :], lhsT=onesw[:, 0:1], rhs=usb[:, :],
                start=True, stop=True,
            )

        # software-pipelined; load issues interleave with compute emission
        # so no latency-critical op queues behind a stalled dma_start.
        def scores_a(b, sbc, ec):
            for h in range(2):
                scores_a_half(b, h, sbc, ec)

        ecs = []
        for _b in range(BPC):
            ec_t = ecpool.tile([PT, TILES], bf16, tag="ec")
            ecs.append(ec_t)
        load_p8t(0)
        load_p8t(1)
        load_p(0)
        sbc0 = scores_b_part(0)
        load_p8t(2)
        load_p(1)
        sbc1 = scores_b_part(1)
        scores_a(0, sbc0, ecs[0])
        pooling(0, ecs[0])
        load_p8t(3)
        load_p(2)
        sbc2 = scores_b_part(2)
        scores_a(1, sbc1, ecs[1])
        pooling(1, ecs[1])
        load_p(3)
        sbc3 = scores_b_part(3)
        scores_a(2, sbc2, ecs[2])
        pooling(2, ecs[2])
        scores_a(3, sbc3, ecs[3])
        pooling(3, ecs[3])

    _legalize_waits(nc)
    _merge_sem_updates(nc)
    return nc


def kernel(Q, W, mask, kernel, bias):
    """Full unsharded inputs -> full [B, D] float32 output. W/bias are
    mathematically irrelevant (per-batch additive constant cancels in
    softmax), so they are not shipped to the device."""
    global LAST_RESULT
    import ml_dtypes
    from concourse.bass_utils import run_bass_kernel_spmd

    trace = os.environ.get("KERNEL_TRACE", "0") == "1"
    if trace:
        _install_ntff_shim()

    if "nc" not in _CACHE:
        _CACHE["nc"] = _build()
    nc = _CACHE["nc"]

    Q = np.asarray(Q, dtype=np.float32)
    mask_f = np.asarray(mask).astype(np.float32)
    kq = np.asarray(kernel, dtype=np.float32)[:D, 0]            # [256]
    # column order: top-|kq| block first (scored exactly on DVE from bf16),
    # bottom block second (scored on PE from the fp8 transposed copy --
    # its quantization noise is scaled by the small kq's, ~2.4x less error
    # than quantizing all columns).  The pooling output is un-permuted on
    # the host after gather.
    order = np.argsort(-np.abs(kq), kind="stable")
    kq_o = kq[order]
    inv_kq = np.where(kq_o == 0.0, 0.0, 1.0 / np.where(kq_o == 0.0, 1.0, kq_o))
    inv_kq = np.ascontiguousarray(inv_kq.reshape(1, D), dtype=np.float32)

    Pm = Q[:, :, order] * kq_o[None, None, :] * mask_f[:, :, None]  # [B, L, D]

    P = np.empty((B, L, DP), dtype=np.float32)
    P[:, :, :D] = Pm
    P[:, :, D] = mask_f
    P[:, :, D + 1] = 0.0
    P = P.astype(ml_dtypes.bfloat16)
    # [core, batch, partition, tile, d] with l = tile*128 + partition
    ps = P.reshape(NCORES, BPC, TILES, PT, DP).transpose(0, 1, 3, 2, 4)

    # P8T blocks for the M=4 indicator score matmuls:
    # X[b, k, j, p, n] = S8*Pm[b, l, d] with d = 32j + p%32 and
    # l' = k*2048 + (p//32)*512 + n, l = (l'%32)*128 + l'//32  (l' = p*32+t)
    A2 = Pm[:, :, DA:] * S8                                     # [B, L, DB]
    kk = np.arange(2)[None, :, None]
    pp = np.arange(PT)[:, None, None]
    nn = np.arange(512)[None, None, :]
    lprime = (pp // 32) * 1024 + kk * 512 + nn                  # [128,2,512]
    lmap = (lprime % TILES) * PT + lprime // TILES
    dmap = (32 * np.arange(4)[:, None] + np.arange(PT)[None, :] % 32)  # [4,128]
    # X[b, p, k, j, n] = S8*Pm_B[b, l(lprime[p,k,n]), dmap[j,p]]
    X = A2[:, lmap[:, :, None, :], dmap.T[:, None, :, None]]    # [B,128,2,4,512]
    p8s = X.astype(ml_dtypes.float8_e4m3).reshape(NCORES, BPC, PT, 2, 4, 512)

    ind8 = np.zeros((PT, 4), dtype=ml_dtypes.float8_e4m3)
    for m in range(4):
        ind8[32 * m:32 * (m + 1), m] = 1.0
    onesw = np.ones((PT, 1), dtype=np.float32)

    in_maps = []
    for i in range(NCORES):
        in_maps.append(
            {
                "p": np.ascontiguousarray(ps[i]),
                "p8t": np.ascontiguousarray(p8s[i]),
                "invkq": inv_kq,
                "ind8": ind8,
                "onesw": onesw,
            }
        )

    res = run_bass_kernel_spmd(
        nc,
        in_maps,
        core_ids=list(range(NCORES)),
        trace=trace,
        tmpdir=os.environ.get("KERNEL_TRACE_DIR") or None,
    )
    LAST_RESULT = res
    out_p = np.concatenate([res.results[i]["out"] for i in range(NCORES)], axis=0)
    out = np.empty_like(out_p)
    out[:, order] = out_p
    return out.astype(np.float32)


# revision 21
# speedup vs baseline: 1.3884x; 1.0041x over previous
"""Trainium2 Bass kernel for masked softmax attention-pooling.

Reference computation (per batch b):
    scores[l] = Q[b,l,:] . kernel[:D,0]  (+ const_b, which cancels in softmax)
    alpha     = softmax_l(scores masked by mask[b])
    out[b,:]  = sum_l alpha[l] * Q[b,l,:]

Distribution: pure data parallel, 4 batches per core across 8 NeuronCores.

Design: BOTH reductions run on the TensorEngine, which streams operands at
128 elem/cycle -- the DVE/ScalarE row-sum path (tensor_reduce is hard-capped
at 1 elem/lane/cycle @0.96 GHz) cannot keep up with the DMA stream and was
the bottleneck of all earlier variants.  Two shipments of the data:

  - P   [l-partitions, d-free] bf16: pooling operand (exactly-precise path).
    Rows pre-scaled by kq (undone by a 1/kq epilogue multiply), masked rows
    zeroed, mask column appended (doubles as the Z accumulator in the
    pooling matmul: masked rows contribute exp(0)*0 = 0 to U and Z).
  - P8T [d-partitions, l-free] fp8e4m3 (x64 scale): score operand.
    scores come from ones-weight matmuls contracting d over the partitions;
    fp8 keeps this copy at half weight (score quantization noise ~1.4e-2,
    inside the 2e-2 gate with the bf16-exact pooling path).

Score path per batch: 8 matmul groups (512 l-columns each, 2 d-halves
accumulated) write S*s into PSUM spread over partitions {0,32,64,96} x 2
banks via column tile_position; ScalarE exp(s/S) reads the 4 partitions
strided and a 4-descriptor SBUF->SBUF scatter DMA per group-pair transposes
e from [4 rows, 1024] into the [128, 32] column layout the pooling matmuls
need as stationary weights (the host orders P8T columns l' = p*32 + t to
make every scatter run contiguous).  Pooling matmuls are column-tiled 2x
(tiles alternate PE column groups 0/32) and merged by a tiny ones-weight
f32 matmul per batch; epilogue out = U * (1/Z) * (1/kq) in one fused
VectorE op.  DMA: all bulk loads queued up-front alternating the two HWDGE
rings (~210 GB/s each); per-batch order p8t then 4 P chunks, so each
batch's score chain completes while its pooling data streams in.
"""

import os

import numpy as np

B, L, D = 32, 4096, 256
DP = D + 2                 # +mask column (doubles as Z accumulator), +pad
NCORES = 8
BPC = B // NCORES          # batches per core
PT = 128                   # partition tile (l rows per tile)
TILES = L // PT            # 32 l-tiles per batch
CHUNK = 8                  # l-tiles per pooling DMA/matmul group
NCHUNK = TILES // CHUNK
S8 = 256.0                 # fp8 pre-scale for the low-|kq| column block:
                           # |Q*kq_lo| <~ 0.25 -> |P8T| <~ 64 < 448, keeps
                           # small values out of the subnormal floor
DA = 128                   # columns scored on DVE (top |kq|, from bf16 P)
DB = D - DA                # columns scored on PE (bottom |kq|, fp8 P8T)

_CACHE = {}
LAST_RESULT = None


def _install_ntff_shim():
    """Register the missing antenv.axon_hooks module so trace=True works."""
    import sys
    import types

    if "antenv.axon_hooks" in sys.modules:
        return
    mod = types.ModuleType("antenv.axon_hooks")
    state = {"hook": None}

    def set_axon_ntff_profile_hook(h):
        state["hook"] = h

    def get_axon_ntff_profile_hook():
        return state["hook"]

    mod.set_axon_ntff_profile_hook = set_axon_ntff_profile_hook
    mod.get_axon_ntff_profile_hook = get_axon_ntff_profile_hook
    sys.modules["antenv.axon_hooks"] = mod
    try:
        import antenv

        antenv.axon_hooks = mod
        from trn_agent_boot.trn_boot import _ntff_profile_via_ctypes

        set_axon_ntff_profile_hook(_ntff_profile_via_ctypes("/opt/axon/libaxon_pjrt.so"))
    except Exception:
        pass


def _legalize_waits(nc):
    """This walrus build accepts at most one sync wait per instruction.
    Tile emits several on some instructions; move the extras onto injected
    NOPs on the same engine immediately before the instruction (engine
    streams execute in block order, so the waits still happen-before)."""
    from concourse import mybir

    counter = [0]
    for fn in nc.m.functions:
        for bb in fn.blocks:
            insts = bb.instructions
            i = 0
            while i < len(insts):
                inst = insts[i]
                si = inst.sync_info
                waits = list(si.on_wait) if si and si.on_wait else []
                if len(waits) > 1:
                    si.on_wait = [waits[0]]
                    for w in waits[1:]:
                        counter[0] += 1
                        nop = mybir.InstNoOp(
                            name=f"legalize-wait-{counter[0]}", ins=[], outs=[]
                        )
                        nop.engine = inst.engine
                        nop.sync_info = mybir.SyncInfo(on_wait=[w], on_update=[])
                        insts.insert(i, nop)
                        i += 1
                i += 1


def _merge_sem_updates(nc):
    """Each instruction-attached sem increment lowers to a serialized EVT_SEM
    write on the issuing engine (~50-115 ns); with 200+ matmuls the PE pays
    several us for these at the kernel tail. walrus requires UpdateValue == 1,
    so instead of merging values we DROP every increment whose running count
    is never awaited and rebase all wait thresholds to their rank among the
    kept increments -- the waiter still unblocks on completion of exactly the
    same producer instruction."""
    from concourse import mybir

    skip_types = ("InstDMACopy", "InstEventSemaphore", "InstDrain", "InstISA")
    blocks = [bb for fn in nc.m.functions for bb in fn.blocks]

    awaited = {}
    sem_info = {}
    for bb in blocks:
        for inst in bb.instructions:
            si = inst.sync_info
            if si is None:
                continue
            for w in si.on_wait or []:
                if (
                    w.sync_type != "semaphore"
                    or w.wait_mode != "sem-ge-imm"
                    or w.wait_reg is not None
                ):
                    sem_info[w.id] = None  # unknown semantics; leave alone
                    continue
                awaited.setdefault(w.id, set()).add(w.wait_value)
            for u in si.on_update or []:
                if u.sync_type != "semaphore":
                    continue
                info = sem_info.setdefault(u.id, {"engine": inst.engine, "ok": True})
                if info is None:
                    continue
                if (
                    u.update_mode != "sem-inc"
                    or u.update_value != 1
                    or u.update_reg is not None
                    or inst.engine != info["engine"]
                    or type(inst).__name__ in skip_types
                ):
                    info["ok"] = False

    mergeable = {
        sid
        for sid, info in sem_info.items()
        if info is not None and info["ok"] and awaited.get(sid)
    }

    for sid in mergeable:
        targets = awaited[sid]
        rank = {v: i + 1 for i, v in enumerate(sorted(targets))}
        cum = 0
        for bb in blocks:
            for inst in bb.instructions:
                si = inst.sync_info
                if si is None:
                    continue
                if si.on_update:
                    ups = list(si.on_update)
                    changed = False
                    for u in list(ups):
                        if u.sync_type != "semaphore" or u.id != sid:
                            continue
                        cum += 1
                        if cum not in targets:
                            ups = [x for x in ups if x is not u]
                            changed = True
                    if changed:
                        si.on_update = ups
                if si.on_wait:
                    ws = list(si.on_wait)
                    changed = False
                    for i, w in enumerate(ws):
                        if w.sync_type == "semaphore" and w.id == sid:
                            ws[i] = mybir.SyncWait(
                                sync_type="semaphore",
                                id=sid,
                                ant_name=w.ant_name,
                                wait_mode="sem-ge-imm",
                                wait_value=rank[w.wait_value],
                            )
                            changed = True
                    if changed:
                        si.on_wait = ws


def _build():
    from contextlib import ExitStack

    from concourse import bass, mybir, tile

    f32 = mybir.dt.float32
    bf16 = mybir.dt.bfloat16
    fp8 = mybir.dt.float8e4
    Alu = mybir.AluOpType
    Act = mybir.ActivationFunctionType

    nc = bass.Bass("TRN2", debug=False, enable_asserts=False, num_devices=NCORES)
    # P pre-tiled [batch, partition, tile, d]: each partition's chunk is one
    # contiguous run in DRAM -> 128 large descriptors per transfer.
    p_ext = nc.declare_dram_parameter("p", [BPC, PT, TILES, DP], bf16, isOutput=False)
    p8t_ext = nc.declare_dram_parameter("p8t", [BPC, PT, 2, 4, 512], fp8, isOutput=False)
    invkq_ext = nc.declare_dram_parameter("invkq", [1, D], f32, isOutput=False)
    ind8_ext = nc.declare_dram_parameter("ind8", [PT, 4], fp8, isOutput=False)
    onesw_ext = nc.declare_dram_parameter("onesw", [PT, 1], f32, isOutput=False)
    out_ext = nc.declare_dram_parameter("out", [BPC, D], f32, isOutput=True)

    with tile.TileContext(nc) as tc, ExitStack() as ctx:
        consts = ctx.enter_context(tc.tile_pool(name="consts", bufs=1))
        ppool = ctx.enter_context(tc.tile_pool(name="ppool", bufs=BPC))
        p8pool = ctx.enter_context(tc.tile_pool(name="p8pool", bufs=BPC))
        erpool = ctx.enter_context(tc.tile_pool(name="erpool", bufs=2))
        ecpool = ctx.enter_context(tc.tile_pool(name="ecpool", bufs=8))
        hpool = ctx.enter_context(tc.tile_pool(name="hpool", bufs=2))
        spool = ctx.enter_context(tc.tile_pool(name="spool", bufs=4))
        usbp = ctx.enter_context(tc.tile_pool(name="usbp", bufs=1))
        small = ctx.enter_context(tc.tile_pool(name="small", bufs=2))
        spsum = ctx.enter_context(tc.tile_pool(name="spsum", bufs=2, space="PSUM"))
        upsum = ctx.enter_context(tc.tile_pool(name="upsum", bufs=2, space="PSUM"))
        mpsum = ctx.enter_context(tc.tile_pool(name="mpsum", bufs=1, space="PSUM"))
        wpsum = ctx.enter_context(tc.tile_pool(name="wpsum", bufs=1, space="PSUM"))
        ctx.enter_context(
            nc.allow_low_precision(
                reason="e stored bf16 (0.1% on softmax weights) and scores "
                "via fp8 matmuls; both far inside the 2e-2 gate"
            )
        )

        rings = [nc.sync, nc.scalar]

        invkq = consts.tile([1, D], f32, tag="invkq")
        nc.sync.dma_start(out=invkq[:, :], in_=invkq_ext[:, :])
        ind8 = consts.tile([PT, 4], fp8, tag="ind8")
        nc.scalar.dma_start(out=ind8[:, :], in_=ind8_ext[:, :])
        onesw = consts.tile([PT, 1], f32, tag="onesw")
        nc.sync.dma_start(out=onesw[:, :], in_=onesw_ext[:, :])

        # usb: merge-matmul rhs staging.  Zeroed once; per batch only
        # partitions 0-32 are rewritten, the rest stay 0 so the full-ones
        # merge weights see no garbage.
        usb = usbp.tile([PT, D + 1], f32, tag="usb")
        nc.vector.memset(usb[:, :], 0.0)

        # Bulk loads are emitted INTERLEAVED with the compute ops (see the
        # pipeline at the bottom): a dma_start whose DMAHW flow-control lane
        # is still busy stalls the issuing engine, and anything queued
        # behind it (exp!) head-of-line blocks.  Each p8t ships as two
        # ring-parallel halves (the k dim) and each P batch as two 16-tile
        # halves, so both rings stay fed with ~0.5-1 MB transfers.
        p_tiles, p8_tiles = [], []
        for b in range(BPC):
            p8_b = p8pool.tile([PT, 2, 4, 512], fp8, tag="p8t")
            p8_tiles.append(p8_b)
            p_b = ppool.tile([PT, TILES, DP], bf16, tag="P")
            p_tiles.append(p_b)

        def load_p8t(b):
            for k in range(2):
                rings[k].dma_start(
                    out=p8_tiles[b][:, k, :, :], in_=p8t_ext[b, :, k]
                )

        def load_p(b):
            for h in range(2):
                lo, hi = 16 * h, 16 * (h + 1)
                rings[h].dma_start(
                    out=p_tiles[b][:, lo:hi, :], in_=p_ext[b][:, lo:hi, :]
                )

        def scores_b_part(b):
            """Low-|kq| column block: indicator matmuls -> s rows (bf16,
            descaled by 1/S8 in the ACT copy) -> scatter to column layout."""
            p8_b = p8_tiles[b]
            sA = spsum.tile([4, 512], f32, tag="sA")
            sB = spsum.tile([4, 512], f32, tag="sB")
            banks = [sA, sB]
            er = erpool.tile([4, 2 * 512], bf16, tag="erows")
            for k in range(2):
                for j in range(4):
                    nc.tensor.matmul(
                        out=banks[k][:, :],
                        lhsT=ind8[:, :],
                        rhs=p8_b[:, k, j, :],
                        start=(j == 0),
                        stop=(j == 3),
                    )
                nc.scalar.activation(
                    out=er[:, 512 * k:512 * (k + 1)], in_=banks[k][:, :],
                    func=Act.Copy, scale=1.0 / S8,
                )
            sbc = ecpool.tile([PT, TILES], bf16, tag="sbc")
            for m in range(4):
                nc.gpsimd.dma_start(
                    out=sbc[32 * m:32 * (m + 1), :], in_=er[m:m + 1, :]
                )
            return sbc

        def scores_a_half(b, h, sbc, ec):
            """Top-|kq| block scored on DVE straight from bf16 P (columns
            0:DA of each row, split in two packed-add halves), fused with
            the scattered B-part and exp'd into pooling-weight columns."""
            lo, hi = 8 * h, 8 * (h + 1)
            p_b = p_tiles[b]
            ha = hpool.tile([PT, 8, DA // 2], bf16, tag="ha")
            nc.vector.tensor_tensor(
                out=ha[:, :, :],
                in0=p_b[:, lo:hi, 0:DA // 2],
                in1=p_b[:, lo:hi, DA // 2:DA],
                op=Alu.add,
            )
            sa = spool.tile([PT, 8], bf16, tag="sa")
            nc.vector.tensor_reduce(
                out=sa[:, :], in_=ha[:, :, :],
                axis=mybir.AxisListType.X, op=Alu.add,
            )
            st = spool.tile([PT, 8], bf16, tag="st")
            nc.vector.tensor_tensor(
                out=st[:, :], in0=sa[:, :], in1=sbc[:, lo:hi], op=Alu.add,
            )
            nc.scalar.activation(
                out=ec[:, lo:hi], in_=st[:, :], func=Act.Exp
            )

        def pooling(b, ec):
            """Column-tiled 2x pooling matmuls + merge + epilogue."""
            p_b = p_tiles[b]
            u_ps = upsum.tile([33, D + 1], f32, tag="U")
            for t in range(TILES):
                cp = 32 * (t % 2)
                nc.tensor.matmul(
                    out=u_ps[cp:cp + 1, 0:D + 1],
                    lhsT=ec[:, t:t + 1],
                    rhs=p_b[:, t, 0:D + 1],
                    start=(t < 2),
                    stop=(t >= TILES - 2),
                    tile_position=(0, cp),
                )
            # merge the two column-group accumulators: copy to SBUF (only
            # partitions 0-32 written; the rest are the memset zeros) and
            # contract with full-ones f32 weights.
            nc.vector.tensor_copy(out=usb[0:1, :], in_=u_ps[0:1, :])
            nc.vector.tensor_copy(out=usb[32:33, :], in_=u_ps[32:33, :])
            m_ps = mpsum.tile([1, D + 1], f32, tag="M")
            nc.tensor.matmul(
                out=m_ps[:, :], lhsT=onesw[:, 0:1], rhs=usb[:, :],
                start=True, stop=True,
            )
            rz = small.tile([1, 1], f32, tag="rz")
            nc.vector.reciprocal(out=rz[:, :], in_=m_ps[:, D:D + 1])
            osb = small.tile([1, D], f32, tag="osb")
            # out = (U * (1/Z)) * (1/kq), one fused VectorE op
            nc.vector.scalar_tensor_tensor(
                out=osb[:, :],
                in0=m_ps[:, 0:D],
                scalar=rz[:, :],
                in1=invkq[:, :],
                op0=Alu.mult,
                op1=Alu.mult,
            )
            nc.scalar.dma_start(out=out_ext[b:b + 1, :], in_=osb[:, :])

        # software-pipelined; load issues interleave with compute emission
        # so no latency-critical op queues behind a stalled dma_start.
        def scores_a(b, sbc, ec):
            for h in range(4):
                scores_a_half(b, h, sbc, ec)

        ecs = []
        for _b in range(BPC):
            ec_t = ecpool.tile([PT, TILES], bf16, tag="ec")
            ecs.append(ec_t)
        load_p(0)
        load_p8t(0)
        load_p8t(1)
        sbc0 = scores_b_part(0)
        load_p8t(2)
        load_p(1)
        sbc1 = scores_b_part(1)
        scores_a(0, sbc0, ecs[0])
        pooling(0, ecs[0])
        load_p8t(3)
        load_p(2)
        sbc2 = scores_b_part(2)
        scores_a(1, sbc1, ecs[1])
        pooling(1, ecs[1])
        load_p(3)
        sbc3 = scores_b_part(3)
        scores_a(2, sbc2, ecs[2])
        pooling(2, ecs[2])
        scores_a(3, sbc3, ecs[3])
        pooling(3, ecs[3])

    _legalize_waits(nc)
    _merge_sem_updates(nc)
    return nc


def kernel(Q, W, mask, kernel, bias):
    """Full unsharded inputs -> full [B, D] float32 output. W/bias are
    mathematically irrelevant (per-batch additive constant cancels in
    softmax), so they are not shipped to the device."""
    global LAST_RESULT
    import ml_dtypes
    from concourse.bass_utils import run_bass_kernel_spmd

    trace = os.environ.get("KERNEL_TRACE", "0") == "1"
    if trace:
        _install_ntff_shim()

    if "nc" not in _CACHE:
        _CACHE["nc"] = _build()
    nc = _CACHE["nc"]

    Q = np.asarray(Q, dtype=np.float32)
    mask_f = np.asarray(mask).astype(np.float32)
    kq = np.asarray(kernel, dtype=np.float32)[:D, 0]            # [256]
    # column order: top-|kq| block first (scored exactly on DVE from bf16),
    # bottom block second (scored on PE from the fp8 transposed copy --
    # its quantization noise is scaled by the small kq's, ~2.4x less error
    # than quantizing all columns).  The pooling output is un-permuted on
    # the host after gather.
    order = np.argsort(-np.abs(kq), kind="stable")
    kq_o = kq[order]
    inv_kq = np.where(kq_o == 0.0, 0.0, 1.0 / np.where(kq_o == 0.0, 1.0, kq_o))
    inv_kq = np.ascontiguousarray(inv_kq.reshape(1, D), dtype=np.float32)

    Pm = Q[:, :, order] * kq_o[None, None, :] * mask_f[:, :, None]  # [B, L, D]

    P = np.empty((B, L, DP), dtype=np.float32)
    P[:, :, :D] = Pm
    P[:, :, D] = mask_f
    P[:, :, D + 1] = 0.0
    P = P.astype(ml_dtypes.bfloat16)
    # [core, batch, partition, tile, d] with l = tile*128 + partition
    ps = P.reshape(NCORES, BPC, TILES, PT, DP).transpose(0, 1, 3, 2, 4)

    # P8T blocks for the M=4 indicator score matmuls:
    # X[b, k, j, p, n] = S8*Pm[b, l, d] with d = 32j + p%32 and
    # l' = k*2048 + (p//32)*512 + n, l = (l'%32)*128 + l'//32  (l' = p*32+t)
    A2 = Pm[:, :, DA:] * S8                                     # [B, L, DB]
    kk = np.arange(2)[None, :, None]
    pp = np.arange(PT)[:, None, None]
    nn = np.arange(512)[None, None, :]
    lprime = (pp // 32) * 1024 + kk * 512 + nn                  # [128,2,512]
    lmap = (lprime % TILES) * PT + lprime // TILES
    dmap = (32 * np.arange(4)[:, None] + np.arange(PT)[None, :] % 32)  # [4,128]
    # X[b, p, k, j, n] = S8*Pm_B[b, l(lprime[p,k,n]), dmap[j,p]]
    X = A2[:, lmap[:, :, None, :], dmap.T[:, None, :, None]]    # [B,128,2,4,512]
    p8s = X.astype(ml_dtypes.float8_e4m3).reshape(NCORES, BPC, PT, 2, 4, 512)

    ind8 = np.zeros((PT, 4), dtype=ml_dtypes.float8_e4m3)
    for m in range(4):
        ind8[32 * m:32 * (m + 1), m] = 1.0
    onesw = np.ones((PT, 1), dtype=np.float32)

    in_maps = []
    for i in range(NCORES):
        in_maps.append(
            {
                "p": np.ascontiguousarray(ps[i]),
                "p8t": np.ascontiguousarray(p8s[i]),
                "invkq": inv_kq,
                "ind8": ind8,
                "onesw": onesw,
            }
        )

    res = run_bass_kernel_spmd(
        nc,
        in_maps,
        core_ids=list(range(NCORES)),
        trace=trace,
        tmpdir=os.environ.get("KERNEL_TRACE_DIR") or None,
    )
    LAST_RESULT = res
    out_p = np.concatenate([res.results[i]["out"] for i in range(NCORES)], axis=0)
    out = np.empty_like(out_p)
    out[:, order] = out_p
    return out.astype(np.float32)


# revision 22
# speedup vs baseline: 1.7874x; 1.2874x over previous
"""Trainium2 Bass kernel for masked softmax attention-pooling.

Reference computation (per batch b):
    scores[l] = Q[b,l,:] . kernel[:D,0]  (+ const_b, which cancels in softmax)
    alpha     = softmax_l(scores masked by mask[b])
    out[b,:]  = sum_l alpha[l] * Q[b,l,:]

Distribution: pure data parallel, 4 batches per core across 8 NeuronCores.

Sharding prep on host (pure elementwise/layout/dtype transforms): P is Q
diagonally pre-scaled by kq (undone exactly by a 1/kq multiply in the device
epilogue) with two extra columns appended — a ones column, so the TensorE
weighted-sum pass accumulates the softmax normalizer Z for free, and a
log-mask column (0 for kept positions, -60 for masked ones), so the score
reduction directly yields s + 1 + logmask and exp() gives exactly-masked
weights. P ships as bf16 (norm rel err ~1.4e-3, far inside the 2e-2 gate)
pre-tiled [batch, partition, tile, d] so every DMA descriptor covers one
contiguous multi-KiB run. All O(B*L*D) reductions — the score sums, the
softmax, and the weighted sum — run on the NeuronCores:

  - P chunks DMA'd from HBM straight into per-batch SBUF buffers
    (sync + scalar HWDGE queues alternate).
  - Scores: one VectorE 3D tensor_reduce per chunk covers 6-7 of 8 tiles;
    ScalarE picks up the other 1-2 via activation(Copy, accum_out), so the
    two engines finish together just under the DMA roofline.
  - Per chunk: ScalarE exp(s) (softmax is shift invariant and |s| < 8, so
    no max pass is needed and exp cannot overflow; masked scores are ~-60
    and underflow to 0), then 8 TensorE matmuls accumulate
    U'[0:257] = sum_l exp(s_l) * P'[l, 0:257] in PSUM (U'[256] = Z).
  - Epilogue: out = U' * (1/Z) * (1/kq) in one fused VectorE op, DMA out.
"""

import os

import numpy as np

B, L, D = 32, 4096, 256
DP = D + 4                 # +ones column (Z accumulator), +log-mask column
                           # (0 or -60), +2 zero pads: 260 = 2*130, so the
                           # row splits into two 4-byte-aligned 130-col halves
NCORES = 8
BPC = B // NCORES          # batches per core
PT = 128                   # partition tile (l rows per tile)
TILES = L // PT            # 32 l-tiles per batch
CHUNK = 8                  # l-tiles per exp/mask/matmul group
NCHUNK = TILES // CHUNK
HD = DP // 2               # half-row width for the two-stage score reduce

_CACHE = {}
LAST_RESULT = None


def _install_ntff_shim():
    """Register the missing antenv.axon_hooks module so trace=True works."""
    import sys
    import types

    if "antenv.axon_hooks" in sys.modules:
        return
    mod = types.ModuleType("antenv.axon_hooks")
    state = {"hook": None}

    def set_axon_ntff_profile_hook(h):
        state["hook"] = h

    def get_axon_ntff_profile_hook():
        return state["hook"]

    mod.set_axon_ntff_profile_hook = set_axon_ntff_profile_hook
    mod.get_axon_ntff_profile_hook = get_axon_ntff_profile_hook
    sys.modules["antenv.axon_hooks"] = mod
    try:
        import antenv

        antenv.axon_hooks = mod
        from trn_agent_boot.trn_boot import _ntff_profile_via_ctypes

        set_axon_ntff_profile_hook(_ntff_profile_via_ctypes("/opt/axon/libaxon_pjrt.so"))
    except Exception:
        pass


def _legalize_waits(nc):
    """This walrus build accepts at most one sync wait per instruction.
    Tile emits several on some instructions; move the extras onto injected
    NOPs on the same engine immediately before the instruction (engine
    streams execute in block order, so the waits still happen-before)."""
    from concourse import mybir

    counter = [0]
    for fn in nc.m.functions:
        for bb in fn.blocks:
            insts = bb.instructions
            i = 0
            while i < len(insts):
                inst = insts[i]
                si = inst.sync_info
                waits = list(si.on_wait) if si and si.on_wait else []
                if len(waits) > 1:
                    si.on_wait = [waits[0]]
                    for w in waits[1:]:
                        counter[0] += 1
                        nop = mybir.InstNoOp(
                            name=f"legalize-wait-{counter[0]}", ins=[], outs=[]
                        )
                        nop.engine = inst.engine
                        nop.sync_info = mybir.SyncInfo(on_wait=[w], on_update=[])
                        insts.insert(i, nop)
                        i += 1
                i += 1


def _merge_sem_updates(nc):
    """Each instruction-attached sem increment lowers to a serialized EVT_SEM
    write on the issuing engine (~50-115 ns); with 128 matmuls the PE pays
    ~5 us for these at the kernel tail. walrus requires UpdateValue == 1, so
    instead of merging values we DROP every increment whose running count is
    never awaited and rebase all wait thresholds to their rank among the
    kept increments — the waiter still unblocks on completion of exactly the
    same producer instruction."""
    from concourse import mybir

    skip_types = ("InstDMACopy", "InstEventSemaphore", "InstDrain", "InstISA")
    blocks = [bb for fn in nc.m.functions for bb in fn.blocks]

    awaited = {}
    sem_info = {}
    for bb in blocks:
        for inst in bb.instructions:
            si = inst.sync_info
            if si is None:
                continue
            for w in si.on_wait or []:
                if (
                    w.sync_type != "semaphore"
                    or w.wait_mode != "sem-ge-imm"
                    or w.wait_reg is not None
                ):
                    sem_info[w.id] = None  # unknown semantics; leave alone
                    continue
                awaited.setdefault(w.id, set()).add(w.wait_value)
            for u in si.on_update or []:
                if u.sync_type != "semaphore":
                    continue
                info = sem_info.setdefault(u.id, {"engine": inst.engine, "ok": True})
                if info is None:
                    continue
                if (
                    u.update_mode != "sem-inc"
                    or u.update_value != 1
                    or u.update_reg is not None
                    or inst.engine != info["engine"]
                    or type(inst).__name__ in skip_types
                ):
                    info["ok"] = False

    mergeable = {
        sid
        for sid, info in sem_info.items()
        if info is not None and info["ok"] and awaited.get(sid)
    }

    for sid in mergeable:
        targets = awaited[sid]
        rank = {v: i + 1 for i, v in enumerate(sorted(targets))}
        cum = 0
        for bb in blocks:
            for inst in bb.instructions:
                si = inst.sync_info
                if si is None:
                    continue
                if si.on_update:
                    ups = list(si.on_update)
                    changed = False
                    for u in list(ups):
                        if u.sync_type != "semaphore" or u.id != sid:
                            continue
                        cum += 1
                        if cum not in targets:
                            ups = [x for x in ups if x is not u]
                            changed = True
                    if changed:
                        si.on_update = ups
                if si.on_wait:
                    ws = list(si.on_wait)
                    changed = False
                    for i, w in enumerate(ws):
                        if w.sync_type == "semaphore" and w.id == sid:
                            ws[i] = mybir.SyncWait(
                                sync_type="semaphore",
                                id=sid,
                                ant_name=w.ant_name,
                                wait_mode="sem-ge-imm",
                                wait_value=rank[w.wait_value],
                            )
                            changed = True
                    if changed:
                        si.on_wait = ws


def _build():
    from contextlib import ExitStack

    from concourse import bass, mybir, tile


    f32 = mybir.dt.float32
    pdt = mybir.dt.bfloat16
    Alu = mybir.AluOpType
    Act = mybir.ActivationFunctionType

    nc = bass.Bass("TRN2", debug=False, enable_asserts=False, num_devices=NCORES)
    # P is shipped pre-tiled [batch, partition, tile, d]: each partition's
    # chunk is one contiguous run in DRAM, so the HWDGE emits 128 large
    # descriptors per transfer instead of thousands of 514 B ones.
    p_ext = nc.declare_dram_parameter("p", [BPC, PT, TILES, DP], pdt, isOutput=False)
    invkq_ext = nc.declare_dram_parameter("invkq", [1, D], f32, isOutput=False)
    out_ext = nc.declare_dram_parameter("out", [BPC, D], f32, isOutput=True)

    with tile.TileContext(nc) as tc, ExitStack() as ctx:
        ctx.enter_context(
            nc.allow_low_precision(
                reason="scores accumulated to bf16: keeps the DVE reduce in "
                "the packed fast path; |s|<8 so the ~0.1% bf16 rounding on "
                "exp(s) is far inside the 2e-2 accuracy gate"
            )
        )
        consts = ctx.enter_context(tc.tile_pool(name="consts", bufs=1))
        # All four batches' P buffers coexist (no DMA ever queue-blocks the
        # sync engine waiting on a slot release).
        ppool = ctx.enter_context(tc.tile_pool(name="ppool", bufs=BPC))
        spool = ctx.enter_context(tc.tile_pool(name="spool", bufs=4))
        scr = ctx.enter_context(tc.tile_pool(name="scr", bufs=2))
        hpool = ctx.enter_context(tc.tile_pool(name="hpool", bufs=3))
        small = ctx.enter_context(tc.tile_pool(name="small", bufs=2))
        psum = ctx.enter_context(tc.tile_pool(name="psum", bufs=4, space="PSUM"))

        dma_engines = [nc.sync, nc.scalar]

        p_tiles = []
        for b in range(BPC):
            pv = p_ext[b]  # [128, 32, 258]
            p_b = ppool.tile([PT, TILES, DP], pdt, tag="P")
            p_tiles.append(p_b)
            # Early batches land in 4 smaller DMAs so compute starts
            # sooner; later batches use fewer, larger transfers. Both HWDGE
            # rings are needed: one ring sustains only about half the HBM
            # bandwidth.
            n_dma = 4 if b <= 1 else 2
            step = TILES // n_dma
            for dc in range(n_dma):
                lo, hi = dc * step, (dc + 1) * step
                eng = dma_engines[(b + dc) % 2]
                eng.dma_start(out=p_b[:, lo:hi, :], in_=pv[:, lo:hi, :])

        invkq = consts.tile([1, D], f32, tag="invkq")
        nc.sync.dma_start(out=invkq[:, :], in_=invkq_ext[:, :])

        for b in range(BPC):
            p_b = p_tiles[b]
            s_b = spool.tile([PT, TILES], pdt, tag="s")
            e_b = spool.tile([PT, TILES], pdt, tag="e")
            u_ps = psum.tile([1, DP], f32, tag="U")
            # The very last chunk is processed as two 4-tile halves so the
            # final exp->matmul->epilogue chain is half as long.
            groups = [(c * CHUNK, (c + 1) * CHUNK) for c in range(NCHUNK)]
            if b == BPC - 1:
                groups = groups[:-1] + [
                    (TILES - CHUNK, TILES - CHUNK // 2),
                    (TILES - CHUNK // 2, TILES),
                ]
            exp_lo = 0
            for c, (lo, hi) in enumerate(groups):
                n_act = 1 if c % 4 >= 3 else 2
                nv = (hi - lo) - n_act
                # Two-stage score reduce over the full 260-wide rows (the
                # ones column adds a uniform +1, softmax-cancelled; the
                # log-mask column lands in the sum; pads add 0). Stage 1
                # adds the two 130-col halves in DVE 2x packed-bf16 mode,
                # stage 2 reduces the half-width result.
                h = hpool.tile([PT, CHUNK, HD], pdt, tag="h")
                nc.vector.tensor_tensor(
                    out=h[:, 0:nv, :],
                    in0=p_b[:, lo:lo + nv, 0:HD],
                    in1=p_b[:, lo:lo + nv, HD:DP],
                    op=Alu.add,
                )
                nc.vector.tensor_reduce(
                    out=s_b[:, lo:lo + nv],
                    in_=h[:, 0:nv, :],
                    axis=mybir.AxisListType.X,
                    op=Alu.add,
                )
                if n_act:
                    sc = scr.tile([PT, 2, DP], pdt, tag="scr")
                    for j in range(n_act):
                        t = lo + nv + j
                        nc.scalar.activation(
                            out=sc[:, j, :],
                            in_=p_b[:, t, :],
                            func=Act.Copy,
                            accum_out=s_b[:, t:t + 1],
                        )
                # exp/matmuls fire per PAIR of score groups: halves the
                # number of ScalarE exp instructions (~224-cycle init each)
                # while the score reduces still pipeline per 8 tiles. The
                # log-mask column made s = score + 1 - 60*(1-mask); exp
                # yields the exactly-masked unnormalized weights directly.
                if c % 2 == 1 or c == len(groups) - 1:
                    nc.scalar.activation(
                        out=e_b[:, exp_lo:hi], in_=s_b[:, exp_lo:hi], func=Act.Exp
                    )
                    for t in range(exp_lo, hi):
                        lhsT = e_b[:, t:t + 1]
                        rhs = p_b[:, t, 0:D + 1]
                        nc.tensor.matmul(
                            out=u_ps[:, 0:D + 1],
                            lhsT=lhsT,
                            rhs=rhs,
                            start=(t == 0),
                            stop=(t == TILES - 1),
                        )
                    exp_lo = hi
            rz = small.tile([1, 1], f32, tag="rz")
            nc.vector.reciprocal(out=rz[:, :], in_=u_ps[:, D:D + 1])
            osb = small.tile([1, D], f32, tag="osb")
            # out = (U * (1/Z)) * (1/kq), one fused VectorE op
            nc.vector.scalar_tensor_tensor(
                out=osb[:, :],
                in0=u_ps[:, 0:D],
                scalar=rz[:, :],
                in1=invkq[:, :],
                op0=Alu.mult,
                op1=Alu.mult,
            )
            nc.sync.dma_start(out=out_ext[b:b + 1, :], in_=osb[:, :])

    _legalize_waits(nc)
    _merge_sem_updates(nc)
    return nc


def kernel(Q, W, mask, kernel, bias):
    """Full unsharded inputs -> full [B, D] float32 output. W/bias are
    mathematically irrelevant (per-batch additive constant cancels in
    softmax), so they are not shipped to the device."""
    global LAST_RESULT
    import ml_dtypes
    from concourse.bass_utils import run_bass_kernel_spmd

    trace = os.environ.get("KERNEL_TRACE", "0") == "1"
    if trace:
        _install_ntff_shim()

    if "nc" not in _CACHE:
        _CACHE["nc"] = _build()
    nc = _CACHE["nc"]

    Q = np.asarray(Q, dtype=np.float32)
    mask_f = np.asarray(mask).astype(np.float32)
    kq = np.asarray(kernel, dtype=np.float32)[:D, 0]            # [256]
    inv_kq = np.where(kq == 0.0, 0.0, 1.0 / np.where(kq == 0.0, 1.0, kq))
    inv_kq = np.ascontiguousarray(inv_kq.reshape(1, D), dtype=np.float32)

    P = np.empty((B, L, DP), dtype=np.float32)
    P[:, :, :D] = Q * kq[None, None, :]
    P[:, :, D] = 1.0
    P[:, :, D + 1] = np.where(mask_f > 0.5, 0.0, -60.0)
    P[:, :, D + 2:] = 0.0
    P = P.astype(ml_dtypes.bfloat16)
    # [core, batch, partition, tile, d] with l = tile*128 + partition
    ps = P.reshape(NCORES, BPC, TILES, PT, DP).transpose(0, 1, 3, 2, 4)

    in_maps = []
    for i in range(NCORES):
        in_maps.append(
            {
                "p": np.ascontiguousarray(ps[i]),
                "invkq": inv_kq,
            }
        )

    res = run_bass_kernel_spmd(
        nc,
        in_maps,
        core_ids=list(range(NCORES)),
        trace=trace,
        tmpdir=os.environ.get("KERNEL_TRACE_DIR") or None,
    )
    LAST_RESULT = res
    out = np.concatenate([res.results[i]["out"] for i in range(NCORES)], axis=0)
    return out.astype(np.float32)



# revision 23
# speedup vs baseline: 1.8111x; 1.0133x over previous
"""Trainium2 Bass kernel for masked softmax attention-pooling.

Reference computation (per batch b):
    scores[l] = Q[b,l,:] . kernel[:D,0]  (+ const_b, which cancels in softmax)
    alpha     = softmax_l(scores masked by mask[b])
    out[b,:]  = sum_l alpha[l] * Q[b,l,:]

Distribution: pure data parallel, 4 batches per core across 8 NeuronCores.

Sharding prep on host (pure elementwise/layout/dtype transforms): P is Q
diagonally pre-scaled by kq (undone exactly by a 1/kq multiply in the device
epilogue) with two extra columns appended — a ones column, so the TensorE
weighted-sum pass accumulates the softmax normalizer Z for free, and a
log-mask column (0 for kept positions, -60 for masked ones), so the score
reduction directly yields s + 1 + logmask and exp() gives exactly-masked
weights. P ships as bf16 (norm rel err ~1.4e-3, far inside the 2e-2 gate)
pre-tiled [batch, partition, tile, d] so every DMA descriptor covers one
contiguous multi-KiB run. All O(B*L*D) reductions — the score sums, the
softmax, and the weighted sum — run on the NeuronCores:

  - P chunks DMA'd from HBM straight into per-batch SBUF buffers
    (sync + scalar HWDGE queues alternate).
  - Scores: one VectorE 3D tensor_reduce per chunk covers 6-7 of 8 tiles;
    ScalarE picks up the other 1-2 via activation(Copy, accum_out), so the
    two engines finish together just under the DMA roofline.  Scores
    accumulate to bf16 (not f32): this keeps the DVE reduce in its packed
    fast path (measured 1215 -> 758 ns per chunk) and speeds the ScalarE
    accum reads; |s| < 8 so bf16 rounding costs ~0.1% on exp(s), far
    inside the 2e-2 gate (measured rel err 3.3e-3 vs 1.7e-3 at f32).
  - Per chunk: ScalarE exp(s) (softmax is shift invariant and |s| < 8, so
    no max pass is needed and exp cannot overflow; masked scores are ~-60
    and underflow to 0), then 8 TensorE matmuls accumulate
    U'[0:257] = sum_l exp(s_l) * P'[l, 0:257] in PSUM (U'[256] = Z).
  - Epilogue: out = U' * (1/Z) * (1/kq) in one fused VectorE op, DMA out.
"""

import os

import numpy as np

B, L, D = 32, 4096, 256
DP = D + 4                 # +ones column (Z accumulator), +log-mask column
                           # (0 or -60), +2 zero pads: 260 = 2*130, so the
                           # row splits into two 4-byte-aligned 130-col halves
NCORES = 8
BPC = B // NCORES          # batches per core
PT = 128                   # partition tile (l rows per tile)
TILES = L // PT            # 32 l-tiles per batch
CHUNK = 8                  # l-tiles per exp/mask/matmul group
NCHUNK = TILES // CHUNK
HD = DP // 2               # half-row width for the two-stage score reduce

_CACHE = {}
LAST_RESULT = None


def _install_ntff_shim():
    """Register the missing antenv.axon_hooks module so trace=True works."""
    import sys
    import types

    if "antenv.axon_hooks" in sys.modules:
        return
    mod = types.ModuleType("antenv.axon_hooks")
    state = {"hook": None}

    def set_axon_ntff_profile_hook(h):
        state["hook"] = h

    def get_axon_ntff_profile_hook():
        return state["hook"]

    mod.set_axon_ntff_profile_hook = set_axon_ntff_profile_hook
    mod.get_axon_ntff_profile_hook = get_axon_ntff_profile_hook
    sys.modules["antenv.axon_hooks"] = mod
    try:
        import antenv

        antenv.axon_hooks = mod
        from trn_agent_boot.trn_boot import _ntff_profile_via_ctypes

        set_axon_ntff_profile_hook(_ntff_profile_via_ctypes("/opt/axon/libaxon_pjrt.so"))
    except Exception:
        pass


def _legalize_waits(nc):
    """This walrus build accepts at most one sync wait per instruction.
    Tile emits several on some instructions; move the extras onto injected
    NOPs on the same engine immediately before the instruction (engine
    streams execute in block order, so the waits still happen-before)."""
    from concourse import mybir

    counter = [0]
    for fn in nc.m.functions:
        for bb in fn.blocks:
            insts = bb.instructions
            i = 0
            while i < len(insts):
                inst = insts[i]
                si = inst.sync_info
                waits = list(si.on_wait) if si and si.on_wait else []
                if len(waits) > 1:
                    si.on_wait = [waits[0]]
                    for w in waits[1:]:
                        counter[0] += 1
                        nop = mybir.InstNoOp(
                            name=f"legalize-wait-{counter[0]}", ins=[], outs=[]
                        )
                        nop.engine = inst.engine
                        nop.sync_info = mybir.SyncInfo(on_wait=[w], on_update=[])
                        insts.insert(i, nop)
                        i += 1
                i += 1


def _merge_sem_updates(nc):
    """Each instruction-attached sem increment lowers to a serialized EVT_SEM
    write on the issuing engine (~50-115 ns); with 128 matmuls the PE pays
    ~5 us for these at the kernel tail. walrus requires UpdateValue == 1, so
    instead of merging values we DROP every increment whose running count is
    never awaited and rebase all wait thresholds to their rank among the
    kept increments — the waiter still unblocks on completion of exactly the
    same producer instruction."""
    from concourse import mybir

    skip_types = ("InstDMACopy", "InstEventSemaphore", "InstDrain", "InstISA")
    blocks = [bb for fn in nc.m.functions for bb in fn.blocks]

    awaited = {}
    sem_info = {}
    for bb in blocks:
        for inst in bb.instructions:
            si = inst.sync_info
            if si is None:
                continue
            for w in si.on_wait or []:
                if (
                    w.sync_type != "semaphore"
                    or w.wait_mode != "sem-ge-imm"
                    or w.wait_reg is not None
                ):
                    sem_info[w.id] = None  # unknown semantics; leave alone
                    continue
                awaited.setdefault(w.id, set()).add(w.wait_value)
            for u in si.on_update or []:
                if u.sync_type != "semaphore":
                    continue
                info = sem_info.setdefault(u.id, {"engine": inst.engine, "ok": True})
                if info is None:
                    continue
                if (
                    u.update_mode != "sem-inc"
                    or u.update_value != 1
                    or u.update_reg is not None
                    or inst.engine != info["engine"]
                    or type(inst).__name__ in skip_types
                ):
                    info["ok"] = False

    mergeable = {
        sid
        for sid, info in sem_info.items()
        if info is not None and info["ok"] and awaited.get(sid)
    }

    for sid in mergeable:
        targets = awaited[sid]
        rank = {v: i + 1 for i, v in enumerate(sorted(targets))}
        cum = 0
        for bb in blocks:
            for inst in bb.instructions:
                si = inst.sync_info
                if si is None:
                    continue
                if si.on_update:
                    ups = list(si.on_update)
                    changed = False
                    for u in list(ups):
                        if u.sync_type != "semaphore" or u.id != sid:
                            continue
                        cum += 1
                        if cum not in targets:
                            ups = [x for x in ups if x is not u]
                            changed = True
                    if changed:
                        si.on_update = ups
                if si.on_wait:
                    ws = list(si.on_wait)
                    changed = False
                    for i, w in enumerate(ws):
                        if w.sync_type == "semaphore" and w.id == sid:
                            ws[i] = mybir.SyncWait(
                                sync_type="semaphore",
                                id=sid,
                                ant_name=w.ant_name,
                                wait_mode="sem-ge-imm",
                                wait_value=rank[w.wait_value],
                            )
                            changed = True
                    if changed:
                        si.on_wait = ws


def _build():
    from contextlib import ExitStack

    from concourse import bass, mybir, tile


    f32 = mybir.dt.float32
    pdt = mybir.dt.bfloat16
    Alu = mybir.AluOpType
    Act = mybir.ActivationFunctionType

    nc = bass.Bass("TRN2", debug=False, enable_asserts=False, num_devices=NCORES)
    # P is shipped pre-tiled [batch, partition, tile, d]: each partition's
    # chunk is one contiguous run in DRAM, so the HWDGE emits 128 large
    # descriptors per transfer instead of thousands of 514 B ones.
    p_ext = nc.declare_dram_parameter("p", [BPC, PT, TILES, DP], pdt, isOutput=False)
    invkq_ext = nc.declare_dram_parameter("invkq", [1, D], f32, isOutput=False)
    out_ext = nc.declare_dram_parameter("out", [BPC, D], f32, isOutput=True)

    with tile.TileContext(nc) as tc, ExitStack() as ctx:
        ctx.enter_context(
            nc.allow_low_precision(
                reason="scores accumulated to bf16: keeps the DVE reduce in "
                "the packed fast path; |s|<8 so the ~0.1% bf16 rounding on "
                "exp(s) is far inside the 2e-2 accuracy gate"
            )
        )
        consts = ctx.enter_context(tc.tile_pool(name="consts", bufs=1))
        # All four batches' P buffers coexist (no DMA ever queue-blocks the
        # sync engine waiting on a slot release).
        ppool = ctx.enter_context(tc.tile_pool(name="ppool", bufs=BPC))
        spool = ctx.enter_context(tc.tile_pool(name="spool", bufs=4))
        scr = ctx.enter_context(tc.tile_pool(name="scr", bufs=2))
        hpool = ctx.enter_context(tc.tile_pool(name="hpool", bufs=3))
        small = ctx.enter_context(tc.tile_pool(name="small", bufs=2))
        psum = ctx.enter_context(tc.tile_pool(name="psum", bufs=4, space="PSUM"))

        dma_engines = [nc.sync, nc.scalar]

        p_tiles = []
        for b in range(BPC):
            pv = p_ext[b]  # [128, 32, 258]
            p_b = ppool.tile([PT, TILES, DP], pdt, tag="P")
            p_tiles.append(p_b)
            # Early batches land in 4 smaller DMAs so compute starts
            # sooner; later batches use fewer, larger transfers. Both HWDGE
            # rings are needed: one ring sustains only about half the HBM
            # bandwidth.
            n_dma = 4 if b <= 1 else 2
            step = TILES // n_dma
            for dc in range(n_dma):
                lo, hi = dc * step, (dc + 1) * step
                eng = dma_engines[(b + dc) % 2]
                eng.dma_start(out=p_b[:, lo:hi, :], in_=pv[:, lo:hi, :])

        invkq = consts.tile([1, D], f32, tag="invkq")
        nc.sync.dma_start(out=invkq[:, :], in_=invkq_ext[:, :])

        for b in range(BPC):
            p_b = p_tiles[b]
            s_b = spool.tile([PT, TILES], pdt, tag="s")
            e_b = spool.tile([PT, TILES], pdt, tag="e")
            u_ps = psum.tile([1, DP], f32, tag="U")
            # The very last chunk is processed as two 4-tile halves so the
            # final exp->matmul->epilogue chain is half as long.
            groups = [(c * CHUNK, (c + 1) * CHUNK) for c in range(NCHUNK)]
            if b == BPC - 1:
                groups = groups[:-1] + [
                    (TILES - CHUNK, TILES - CHUNK // 2),
                    (TILES - CHUNK // 2, TILES),
                ]
            exp_lo = 0
            for c, (lo, hi) in enumerate(groups):
                n_act = 1 if c % 4 >= 3 else 2
                nv = (hi - lo) - n_act
                # Two-stage score reduce over the full 260-wide rows (the
                # ones column adds a uniform +1, softmax-cancelled; the
                # log-mask column lands in the sum; pads add 0). Stage 1
                # adds the two 130-col halves in DVE 2x packed-bf16 mode,
                # stage 2 reduces the half-width result.
                h = hpool.tile([PT, CHUNK, HD], pdt, tag="h")
                nc.vector.tensor_tensor(
                    out=h[:, 0:nv, :],
                    in0=p_b[:, lo:lo + nv, 0:HD],
                    in1=p_b[:, lo:lo + nv, HD:DP],
                    op=Alu.add,
                )
                nc.vector.tensor_reduce(
                    out=s_b[:, lo:lo + nv],
                    in_=h[:, 0:nv, :],
                    axis=mybir.AxisListType.X,
                    op=Alu.add,
                )
                if n_act:
                    sc = scr.tile([PT, 2, DP], pdt, tag="scr")
                    for j in range(n_act):
                        t = lo + nv + j
                        nc.scalar.activation(
                            out=sc[:, j, :],
                            in_=p_b[:, t, :],
                            func=Act.Copy,
                            accum_out=s_b[:, t:t + 1],
                        )
                # exp/matmuls fire per PAIR of score groups: halves the
                # number of ScalarE exp instructions (~224-cycle init each)
                # while the score reduces still pipeline per 8 tiles. The
                # log-mask column made s = score + 1 - 60*(1-mask); exp
                # yields the exactly-masked unnormalized weights directly.
                if c % 2 == 1 or c == len(groups) - 1:
                    nc.scalar.activation(
                        out=e_b[:, exp_lo:hi], in_=s_b[:, exp_lo:hi], func=Act.Exp
                    )
                    for t in range(exp_lo, hi):
                        lhsT = e_b[:, t:t + 1]
                        rhs = p_b[:, t, 0:D + 1]
                        nc.tensor.matmul(
                            out=u_ps[:, 0:D + 1],
                            lhsT=lhsT,
                            rhs=rhs,
                            start=(t == 0),
                            stop=(t == TILES - 1),
                        )
                    exp_lo = hi
            rz = small.tile([1, 1], f32, tag="rz")
            nc.vector.reciprocal(out=rz[:, :], in_=u_ps[:, D:D + 1])
            osb = small.tile([1, D], f32, tag="osb")
            # out = (U * (1/Z)) * (1/kq), one fused VectorE op
            nc.vector.scalar_tensor_tensor(
                out=osb[:, :],
                in0=u_ps[:, 0:D],
                scalar=rz[:, :],
                in1=invkq[:, :],
                op0=Alu.mult,
                op1=Alu.mult,
            )
            nc.sync.dma_start(out=out_ext[b:b + 1, :], in_=osb[:, :])

    _legalize_waits(nc)
    _merge_sem_updates(nc)
    return nc


def kernel(Q, W, mask, kernel, bias):
    """Full unsharded inputs -> full [B, D] float32 output. W/bias are
    mathematically irrelevant (per-batch additive constant cancels in
    softmax), so they are not shipped to the device."""
    global LAST_RESULT
    import ml_dtypes
    from concourse.bass_utils import run_bass_kernel_spmd

    trace = os.environ.get("KERNEL_TRACE", "0") == "1"
    if trace:
        _install_ntff_shim()

    if "nc" not in _CACHE:
        _CACHE["nc"] = _build()
    nc = _CACHE["nc"]

    Q = np.asarray(Q, dtype=np.float32)
    mask_f = np.asarray(mask).astype(np.float32)
    kq = np.asarray(kernel, dtype=np.float32)[:D, 0]            # [256]
    inv_kq = np.where(kq == 0.0, 0.0, 1.0 / np.where(kq == 0.0, 1.0, kq))
    inv_kq = np.ascontiguousarray(inv_kq.reshape(1, D), dtype=np.float32)

    P = np.empty((B, L, DP), dtype=np.float32)
    P[:, :, :D] = Q * kq[None, None, :]
    P[:, :, D] = 1.0
    P[:, :, D + 1] = np.where(mask_f > 0.5, 0.0, -60.0)
    P[:, :, D + 2:] = 0.0
    P = P.astype(ml_dtypes.bfloat16)
    # [core, batch, partition, tile, d] with l = tile*128 + partition
    ps = P.reshape(NCORES, BPC, TILES, PT, DP).transpose(0, 1, 3, 2, 4)

    in_maps = []
    for i in range(NCORES):
        in_maps.append(
            {
                "p": np.ascontiguousarray(ps[i]),
                "invkq": inv_kq,
            }
        )

    res = run_bass_kernel_spmd(
        nc,
        in_maps,
        core_ids=list(range(NCORES)),
        trace=trace,
        tmpdir=os.environ.get("KERNEL_TRACE_DIR") or None,
    )
    LAST_RESULT = res
    out = np.concatenate([res.results[i]["out"] for i in range(NCORES)], axis=0)
    return out.astype(np.float32)

